# revision 13
# baseline (speedup 1.0000x reference)
"""DiagonalLSTM Bass/Tile kernel for TRN2 (per-core shard: B=4 images).

Layout "DESIGN-D" (row-contiguous column packing):
  State columns are packed as col = 64*b + p (p = skew-row 0..63, b = image).

  - A2   [128,256] SBUF bf16: parts 0:64  = h_{t-1}[k, (b,p)]
                              parts 64:128= x_t[c, (b,p)] (skewed input col,
                              zero outside the diagonal band)
  - P01/P23 [128,256] PSUM f32: gate preactivations, partition = gate chan
    o%128 (tile w holds chans 128w:128w+128), col = 64b + p_gate.
  - G    [128,512] SBUF: G[64u+k, 128q + 32b + kap] = sigmoid(gate q at
    c-position p2 = 2*kap + u, chan k).  The model's flat-split identity:
    gate q for flat c-position j lives at gate-flat 4096q + j, which maps to
    P tile w = kap%2, col 64b + 16q + kap//2 -- a 3-free-dim AP, so the
    whole P->G permutation is TWO sigmoid ACTIVATEs (vs 4 in the parity
    design).
  - C2n  [128,128]: c state, [64u+k, 32b+kap]; C2e/C2o bf16 casts for the
    c2c matmul rhs (re-based to partition 0).

Per step t: 4 gate matmuls (K-packed [h;x], Ws0 row-shift via offset view),
4 c2c matmuls, 2 sigmoids, full-lane DVE gate math, 1 upsample matmul, 1
OUT bias-add. PE queue runs gate MMs first (critical path), then c2c +
upsample of the previous step under the sigmoids. Output is stored in
staggered 16-row blocks so the final DMA tail is 1/4 of the image.
"""
from contextlib import ExitStack

import numpy as np

import concourse.bass as bass
import concourse.tile as tile
from concourse import bacc, mybir

F32 = mybir.dt.float32
BF = mybir.dt.bfloat16
AF = mybir.ActivationFunctionType
ALU = mybir.AluOpType

B = 4          # images per core
H = 64         # rows
W = 64         # cols
C = 64         # input channels
HID = 64       # hidden
NW = H + W - 1 # 127 diagonal steps

STAGGER_OUT = True


def v(ap, off, dims):
    """Custom view: keep ap's partition dim, replace free dims, add offset
    (in elements)."""
    return bass.AP(ap.tensor, ap.offset + off, [list(ap.ap[0])] + [list(d) for d in dims])


def dv(ap, off, dims):
    """Fully-custom view (DRAM side of DMAs): absolute offset, all dims."""
    return bass.AP(ap.tensor, off, [list(d) for d in dims])


def band(t):
    return max(0, t - (W - 1)), min(H - 1, t)


def build_kernel(ctx, tc, outs, ins):
    nc = tc.nc
    x_d = ins["inputs"]
    out_d = outs["out"]

    const = ctx.enter_context(tc.tile_pool(name="const", bufs=1))
    big = ctx.enter_context(tc.tile_pool(name="big", bufs=1))
    st = ctx.enter_context(tc.tile_pool(name="st", bufs=2))
    tmp = ctx.enter_context(tc.tile_pool(name="tmp", bufs=2))
    ps = ctx.enter_context(tc.tile_pool(name="ps", bufs=2, space="PSUM"))

    # ---------------- weights / biases (one-time prep) ----------------
    # lhsT layouts; matmul computes lhsT.T @ rhs.
    LA01 = const.tile([128, 128], BF, tag="LA01")  # [[Ws1 o=0:128].T ; [Wi2s o=0:128].T]
    LA23 = const.tile([128, 128], BF, tag="LA23")
    LB01 = const.tile([64, 128], BF, tag="LB01")   # Ws0[0:128].T
    LB23 = const.tile([64, 128], BF, tag="LB23")
    LC1 = const.tile([64, 64], BF, tag="LC1")      # Wc1.T
    LC0 = const.tile([64, 64], BF, tag="LC0")
    LU = const.tile([64, 128], BF, tag="LU")       # w_up.T
    LA01f = const.tile([128, 128], F32, tag="LA01f")
    LA23f = const.tile([128, 128], F32, tag="LA23f")
    LB01f = const.tile([64, 128], F32, tag="LB01f")
    LB23f = const.tile([64, 128], F32, tag="LB23f")
    LC1f = const.tile([64, 64], F32, tag="LC1f")
    LC0f = const.tile([64, 64], F32, tag="LC0f")
    LUf = const.tile([64, 128], F32, tag="LUf")
    bi2s = const.tile([128, 2], F32, tag="bi2s")    # col 0: b_i2s, col 1: b_s2s
    bsg01 = const.tile([128, 1], F32, tag="bsg01")
    bi2s_b = const.tile([128, 2], F32, tag="bi2s_b")
    bsg23 = const.tile([128, 1], F32, tag="bsg23")
    bc2c2 = const.tile([128, 1], F32, tag="bc2c2")
    bup = const.tile([128, 1], F32, tag="bup")

    w_s2s = ins["w_s2s"]   # [256, 64, 2] dram
    w_i2s = ins["w_i2s"]   # [256, 64]
    w_c2c = ins["w_c2c"]   # [64, 64, 2]
    w_up = ins["w_up"]     # [128, 64]

    for blk, LA, LB in ((0, LA01f, LB01f), (1, LA23f, LB23f)):
        # LA[kk,m] = Ws1[128*blk+m, kk] (kk<64) | Wi2s[128*blk+m, kk-64]
        nc.sync.dma_start(
            out=LA[0:64, :],
            in_=dv(w_s2s, 128 * blk * 128 + 1, [[2, 64], [128, 128]]),
        )
        nc.sync.dma_start(
            out=LA[64:128, :],
            in_=dv(w_i2s, 128 * blk * 64, [[1, 64], [64, 128]]),
        )
        nc.sync.dma_start(
            out=LB[:, :],
            in_=dv(w_s2s, 128 * blk * 128 + 0, [[2, 64], [128, 128]]),
        )
    nc.sync.dma_start(out=LC1f[:, :], in_=dv(w_c2c, 1, [[2, 64], [128, 64]]))
    nc.sync.dma_start(out=LC0f[:, :], in_=dv(w_c2c, 0, [[2, 64], [128, 64]]))
    nc.sync.dma_start(out=LUf[:, :], in_=dv(w_up, 0, [[1, 64], [64, 128]]))
    for bf_t, f_t in ((LA01, LA01f), (LA23, LA23f), (LB01, LB01f), (LB23, LB23f),
                      (LC1, LC1f), (LC0, LC0f), (LU, LUf)):
        nc.vector.tensor_copy(bf_t[:, :], f_t[:, :])

    b_i2s, b_s2s, b_c2c, b_up = ins["b_i2s"], ins["b_s2s"], ins["b_c2c"], ins["b_up"]
    for blk, (btile, bout) in ((0, (bi2s, bsg01)), (1, (bi2s_b, bsg23))):
        nc.sync.dma_start(out=btile[:, 0:1], in_=dv(b_i2s, 128 * blk, [[1, 128], [1, 1]]))
        nc.sync.dma_start(out=btile[:, 1:2], in_=dv(b_s2s, 128 * blk, [[1, 128], [1, 1]]))
        nc.vector.tensor_add(bout[:, :], btile[:, 0:1], btile[:, 1:2])
    nc.sync.dma_start(out=bc2c2[0:64, :], in_=dv(b_c2c, 0, [[1, 64], [1, 1]]))
    nc.sync.dma_start(out=bc2c2[64:128, :], in_=dv(b_c2c, 0, [[1, 64], [1, 1]]))
    nc.sync.dma_start(out=bup[:, :], in_=dv(b_up, 0, [[1, 128], [1, 1]]))

    # ---------------- input load ----------------
    # IN[c, b*4096 + p*64 + w] = inputs[b, c, p, w]
    IN = big.tile([64, B * H * W], BF, tag="IN")
    for b in range(B):
        nc.sync.dma_start(
            out=IN[:, b * H * W:(b + 1) * H * W],
            in_=dv(x_d, b * C * H * W, [[4096, 64], [1, 4096]]),
        )

    OUT = big.tile([128, B * H * W], F32, tag="OUT")
    IN_ap = IN[:, :]
    OUT_ap = OUT[:, :]

    def xprep(A2b, t):
        """Fill the x half (parts 64:128) of A2b for step t: x[c, 64b+p]
        for p in band(t), zero elsewhere."""
        xa = A2b[64:128, :]
        nc.gpsimd.memset(xa, 0.0)
        lo, hi = band(t)
        n = hi - lo + 1
        nc.gpsimd.tensor_copy(
            out=v(xa, lo, [[64, 4], [1, n]]),
            in_=v(IN_ap, 63 * lo + t, [[4096, 4], [63, n]]),
        )

    # ---------------- initial state ----------------
    A2 = st.tile([128, 256], BF, tag="A2", name="A2", bufs=3)
    nc.gpsimd.memset(A2[0:64, :], 0.0)
    xprep(A2, 0)
    # bf16 halves of c-state (matmul rhs; both re-based to partition 0 --
    # matmul rhs must share the lhsT's base partition)
    C2e = st.tile([64, 128], BF, tag="C2e", bufs=3)
    nc.gpsimd.memset(C2e[:, :], 0.0)
    C2o = st.tile([64, 128], BF, tag="C2o", bufs=3)
    nc.gpsimd.memset(C2o[:, :], 0.0)

    Uprev = None   # (U psum tile, t) pending upsample bias-add
    Hprev = None   # A2 tile holding h_{t-1} (rhs of this step)

    def emit_upsample(A2h, t):
        """Upsample matmul on the in-band part of h_t (held in A2h)."""
        U = ps.tile([128, 256], F32, tag="U")
        lo, hi = band(t)
        n = hi - lo + 1
        nc.tensor.matmul(
            v(U[:, :], lo, [[64, 4], [1, n]]),
            LU[:, :],
            v(A2h[0:64, :], lo, [[64, 4], [1, n]]),
            start=True, stop=True,
        )
        return U

    def emit_outadd(U, t):
        lo, hi = band(t)
        n = hi - lo + 1
        nc.vector.tensor_scalar_add(
            v(OUT_ap, 63 * lo + t, [[4096, 4], [63, n]]),
            v(U[:, :], lo, [[64, 4], [1, n]]),
            bup[:, 0:1],
        )

    def store_block(p0, np_):
        """DMA OUT rows [p0, p0+np_) to DRAM (all images, all channels)."""
        for b in range(B):
            nc.sync.dma_start(
                out=dv(out_d, b * 128 * H * W + p0 * W, [[4096, 128], [1, np_ * W]]),
                in_=OUT[:, b * H * W + p0 * W: b * H * W + (p0 + np_) * W],
            )

    # ---------------- the recurrence ----------------
    for t in range(NW):
        # -- PE: gate matmuls first (critical path) --
        P01 = ps.tile([128, 256], F32, tag="P01")
        P23 = ps.tile([128, 256], F32, tag="P23")
        for P, LA, LB in ((P01, LA01, LB01), (P23, LA23, LB23)):
            nc.tensor.matmul(P[:, :], LA[:, :], A2[:, :], start=True, stop=False)
            # Ws0 row-shift tap: out (b, p>=1) += Ws0 @ h[(b, p-1)]
            nc.tensor.matmul(
                v(P[:, :], 1, [[64, 4], [1, 63]]),
                LB[:, :],
                v(A2[0:64, :], 0, [[64, 4], [1, 63]]),
                start=False, stop=True,
            )

        # -- PE: upsample of the previous step (off critical path) --
        if Hprev is not None:
            Uprev = (emit_upsample(Hprev, t - 1), t - 1)
            Hprev = None

        # -- PE: c2c matmuls (Cp needed only after the sigmoids) --
        Cp = ps.tile([128, 128], F32, tag="Cp")
        nc.tensor.matmul(Cp[0:64, :], LC1[:, :], C2e[:, :], start=True, stop=False,
                         skip_group_check=True)
        nc.tensor.matmul(Cp[64:128, :], LC1[:, :], C2o[:, :], start=True, stop=False,
                         skip_group_check=True)
        # u=1 out += Wc0 @ c-even (same kap)
        nc.tensor.matmul(Cp[64:128, :], LC0[:, :], C2e[:, :], start=False, stop=True,
                         skip_group_check=True)
        # u=0 out (kap>=1) += Wc0 @ c-odd (kap-1)
        nc.tensor.matmul(
            v(Cp[0:64, :], 1, [[32, 4], [1, 31]]),
            LC0[:, :], v(C2o[:, :], 0, [[32, 4], [1, 31]]),
            start=False, stop=True, skip_group_check=True,
        )

        # -- ACT + DVE + Pool: kap-parity split streams --
        # Sigmoid w writes G cols with kap%2 == w, so the even stream's gate
        # math (STT/add on DVE, T1 on Pool) runs in the shadow of sigmoid 2,
        # and tanh_e follows sigmoid 2 back-to-back on ACT. Only the odd
        # stream's tail (STT_o/add_o -> tanh_o -> h-mul_o) is serial.
        G = tmp.tile([128, 512], BF, tag="G")
        Gap = G[:, :]
        T1 = tmp.tile([128, 128], F32, tag="T1")
        T2 = tmp.tile([128, 128], F32, tag="T2")
        C2n = tmp.tile([128, 128], F32, tag="C2n")
        TH = tmp.tile([128, 128], BF, tag="TH")
        A2n = st.tile([128, 256], BF, tag="A2", name="A2n", bufs=3)

        def cs(ap, off):
            """c-space stride-2 kap-parity view ([128, b(4) x kapH(16)])."""
            return v(ap, off, [[32, 4], [2, 16]])

        for w, (P, bsg) in ((0, (P01, bsg01)), (1, (P23, bsg23))):
            nc.scalar.activation(
                v(Gap, w, [[128, 4], [32, 4], [2, 16]]),
                v(P[:, :], 0, [[16, 4], [64, 4], [1, 16]]),
                AF.Sigmoid, bias=bsg[:, 0:1],
            )
        # Pool: T1_w = ig*gg (parallel with DVE's STT_w)
        for w in (0, 1):
            nc.gpsimd.tensor_mul(cs(T1[:, :], w), cs(Gap, 0 + w), cs(Gap, 128 + w))
        # DVE: T2_w = (Cp + b_c2c) * fg -- both parities first, so STT_o
        # issues as soon as sigmoid 2 lands (not behind add_e).
        for w in (0, 1):
            nc.vector.scalar_tensor_tensor(
                out=cs(T2[:, :], w), in0=cs(Cp[:, :], w), scalar=bc2c2[:, 0:1],
                in1=cs(Gap, 256 + w), op0=ALU.add, op1=ALU.mult,
            )
        for w in (0, 1):
            nc.vector.tensor_add(cs(C2n[:, :], w), cs(T1[:, :], w), cs(T2[:, :], w))
            # ACT: tanh_w
            nc.scalar.activation(cs(TH[:, :], w), cs(C2n[:, :], w), AF.Tanh)
        # DVE: h_t = og * tanh(c_t) -> A2n[0:64], col 64b + 4*kapH + 2w + u
        for w in (0, 1):
            for u in (0, 1):
                nc.vector.tensor_mul(
                    v(A2n[0:64, :], 2 * w + u, [[64, 4], [4, 16]]),
                    v(G[64 * u:64 * u + 64, :], 384 + w, [[32, 4], [2, 16]]),
                    v(TH[64 * u:64 * u + 64, :], w, [[32, 4], [2, 16]]),
                )

        # -- GpSimd: x for next step (off the critical chain) --
        if t + 1 < NW:
            xprep(A2n, t + 1)

        # -- DVE (off-chain): bf16 casts of c for next c2c, OUT bias-add --
        C2en = st.tile([64, 128], BF, tag="C2e", bufs=3)
        nc.vector.tensor_copy(C2en[:, :], C2n[0:64, :])
        C2on = st.tile([64, 128], BF, tag="C2o", bufs=3)
        nc.vector.tensor_copy(C2on[:, :], C2n[64:128, :])

        if Uprev is not None:
            emit_outadd(*Uprev)
            Uprev = None

        if STAGGER_OUT and t >= 80 and (t - 80) % 16 == 0 and (t - 80) // 16 < 3:
            store_block(16 * ((t - 80) // 16), 16)

        A2 = A2n
        Hprev = A2n
        C2e = C2en
        C2o = C2on

    # ---------------- epilogue: last upsample + store ----------------
    U = emit_upsample(A2, NW - 1)
    emit_outadd(U, NW - 1)
    if STAGGER_OUT:
        store_block(48, 16)
    else:
        store_block(0, 64)


def build_nc():
    nc = bacc.Bacc("TRN2", target_bir_lowering=False, debug=False)
    ins = {
        "inputs": nc.dram_tensor("inputs", [B, C, H, W], BF, kind="ExternalInput").ap(),
        "w_i2s": nc.dram_tensor("w_i2s", [4 * HID, C], F32, kind="ExternalInput").ap(),
        "b_i2s": nc.dram_tensor("b_i2s", [4 * HID], F32, kind="ExternalInput").ap(),
        "w_s2s": nc.dram_tensor("w_s2s", [4 * HID, HID, 2], F32, kind="ExternalInput").ap(),
        "b_s2s": nc.dram_tensor("b_s2s", [4 * HID], F32, kind="ExternalInput").ap(),
        "w_c2c": nc.dram_tensor("w_c2c", [HID, HID, 2], F32, kind="ExternalInput").ap(),
        "b_c2c": nc.dram_tensor("b_c2c", [HID], F32, kind="ExternalInput").ap(),
        "w_up": nc.dram_tensor("w_up", [2 * HID, HID], F32, kind="ExternalInput").ap(),
        "b_up": nc.dram_tensor("b_up", [2 * HID], F32, kind="ExternalInput").ap(),
    }
    outs = {"out": nc.dram_tensor("out", [B, 2 * HID, H, W], F32, kind="ExternalOutput").ap()}
    with tile.TileContext(nc) as tc:
        with ExitStack() as ctx:
            build_kernel(ctx, tc, outs, ins)
    nc.compile()
    return nc


# ---------------------------------------------------------------------------
# Harness entry point: full inputs -> shard over 8 cores -> full output.
# ---------------------------------------------------------------------------
import ml_dtypes
from concourse.bass_utils import run_bass_kernel_spmd

N_CORES = 8
TRACE = False
LAST_EXEC_NS = None
LAST_RESULT = None
_NC = None


def _get_nc():
    global _NC
    if _NC is None:
        _NC = build_nc()
    return _NC


def kernel(**inputs):
    global LAST_EXEC_NS, LAST_RESULT
    nc = _get_nc()
    full = {k: np.ascontiguousarray(np.asarray(val, np.float32))
            for k, val in inputs.items()}
    xs = full["inputs"].astype(ml_dtypes.bfloat16)
    in_maps = []
    for i in range(N_CORES):
        m = dict(full)
        m["inputs"] = np.ascontiguousarray(xs[B * i:B * (i + 1)])
        in_maps.append(m)
    res = run_bass_kernel_spmd(nc, in_maps, list(range(N_CORES)), trace=TRACE)
    LAST_EXEC_NS = res.exec_time_ns
    LAST_RESULT = res
    return np.concatenate([res.results[i]["out"] for i in range(N_CORES)], axis=0)


# revision 22
# speedup vs baseline: 1.0611x; 1.0611x over previous
"""DiagonalLSTM Bass/Tile kernel for TRN2 (per-core shard: B=4 images).

Layout "DESIGN-E" (contiguous kap-parity split streams):
  State columns (A2 rhs / P gate tiles / U) are packed col = 4*p + b
  (b minor).  c-space tiles (Cp/T1/T2/C2n/TH and the G gate tile) are packed
  kap-parity-major: col = 64*w + 16*b + j where the c position is
  p2 = 2*kap + u, kap = 2*j + w, partition = 64*u + k.

  Why: sigmoid w (reading gate-tile P_w, which holds chans 128w:128w+128 =
  positions with kap%2 == w) then writes the CONTIGUOUS G half
  [256w, 256w+256), so the whole even-parity gate-math stream
  (T1/STT/add on DVE, tanh on ACT) runs on plain [128,64] slices in the
  shadow of sigmoid 2 / tanh_o, with no strided DVE penalty.  Only the odd
  tail (STT_o/add_o -> tanh_o -> h-mul_o) is serial between the sigmoids
  and the next step's gate matmuls.

  G[64u+k, 256w + 64q + 16b + j] = sigmoid(gate q at p2, chan k); the
  model's flat-split identity maps gate q of c-position (p2,k) to
  P_{kap%2} col 4*(16q + j) + b -- a 3-free-dim AP both sides.

Per step: 4 gate matmuls (K-packed [h;x], Ws0 row-shift via offset view),
8 half-width c2c matmuls (w=0 half first so STT_e can start early),
2 sigmoids, contiguous DVE gate math, 1 upsample matmul; the output
bias-add runs on Pool so it cannot block the DVE gate window.  Output is
stored in staggered 16-row blocks so the final DMA tail is 1/4 the image.
"""
from contextlib import ExitStack

import numpy as np

import concourse.bass as bass
import concourse.tile as tile
from concourse import bacc, mybir

F32 = mybir.dt.float32
BF = mybir.dt.bfloat16
AF = mybir.ActivationFunctionType
ALU = mybir.AluOpType

B = 4          # images per core
H = 64         # rows
W = 64         # cols
C = 64         # input channels
HID = 64       # hidden
NW = H + W - 1 # 127 diagonal steps

STAGGER_OUT = True


def v(ap, off, dims):
    """Custom view: keep ap's partition dim, replace free dims, add offset
    (in elements)."""
    return bass.AP(ap.tensor, ap.offset + off, [list(ap.ap[0])] + [list(d) for d in dims])


def dv(ap, off, dims):
    """Fully-custom view (DRAM side of DMAs): absolute offset, all dims."""
    return bass.AP(ap.tensor, off, [list(d) for d in dims])


def band(t):
    return max(0, t - (W - 1)), min(H - 1, t)


def build_kernel(ctx, tc, outs, ins):
    nc = tc.nc
    x_d = ins["inputs"]
    out_d = outs["out"]

    const = ctx.enter_context(tc.tile_pool(name="const", bufs=1))
    big = ctx.enter_context(tc.tile_pool(name="big", bufs=1))
    st = ctx.enter_context(tc.tile_pool(name="st", bufs=2))
    tmp = ctx.enter_context(tc.tile_pool(name="tmp", bufs=2))
    ps = ctx.enter_context(tc.tile_pool(name="ps", bufs=2, space="PSUM"))

    # ---------------- weights / biases (one-time prep) ----------------
    # lhsT layouts; matmul computes lhsT.T @ rhs.
    LA01 = const.tile([128, 128], BF, tag="LA01")  # [[Ws1 o=0:128].T ; [Wi2s o=0:128].T]
    LA23 = const.tile([128, 128], BF, tag="LA23")
    LB01 = const.tile([64, 128], BF, tag="LB01")   # Ws0[0:128].T
    LB23 = const.tile([64, 128], BF, tag="LB23")
    LC1 = const.tile([64, 64], BF, tag="LC1")      # Wc1.T
    LC0 = const.tile([64, 64], BF, tag="LC0")
    LU = const.tile([64, 128], BF, tag="LU")       # w_up.T
    LA01f = const.tile([128, 128], F32, tag="LA01f")
    LA23f = const.tile([128, 128], F32, tag="LA23f")
    LB01f = const.tile([64, 128], F32, tag="LB01f")
    LB23f = const.tile([64, 128], F32, tag="LB23f")
    LC1f = const.tile([64, 64], F32, tag="LC1f")
    LC0f = const.tile([64, 64], F32, tag="LC0f")
    LUf = const.tile([64, 128], F32, tag="LUf")
    bi2s = const.tile([128, 2], F32, tag="bi2s")    # col 0: b_i2s, col 1: b_s2s
    bsg01 = const.tile([128, 1], F32, tag="bsg01")
    bi2s_b = const.tile([128, 2], F32, tag="bi2s_b")
    bsg23 = const.tile([128, 1], F32, tag="bsg23")
    bc2c2 = const.tile([128, 1], F32, tag="bc2c2")
    bup = const.tile([128, 1], F32, tag="bup")

    w_s2s = ins["w_s2s"]   # [256, 64, 2] dram
    w_i2s = ins["w_i2s"]   # [256, 64]
    w_c2c = ins["w_c2c"]   # [64, 64, 2]
    w_up = ins["w_up"]     # [128, 64]

    for blk, LA, LB in ((0, LA01f, LB01f), (1, LA23f, LB23f)):
        # LA[kk,m] = Ws1[128*blk+m, kk] (kk<64) | Wi2s[128*blk+m, kk-64]
        nc.sync.dma_start(
            out=LA[0:64, :],
            in_=dv(w_s2s, 128 * blk * 128 + 1, [[2, 64], [128, 128]]),
        )
        nc.sync.dma_start(
            out=LA[64:128, :],
            in_=dv(w_i2s, 128 * blk * 64, [[1, 64], [64, 128]]),
        )
        nc.sync.dma_start(
            out=LB[:, :],
            in_=dv(w_s2s, 128 * blk * 128 + 0, [[2, 64], [128, 128]]),
        )
    nc.sync.dma_start(out=LC1f[:, :], in_=dv(w_c2c, 1, [[2, 64], [128, 64]]))
    nc.sync.dma_start(out=LC0f[:, :], in_=dv(w_c2c, 0, [[2, 64], [128, 64]]))
    nc.sync.dma_start(out=LUf[:, :], in_=dv(w_up, 0, [[1, 64], [64, 128]]))
    for bf_t, f_t in ((LA01, LA01f), (LA23, LA23f), (LB01, LB01f), (LB23, LB23f),
                      (LC1, LC1f), (LC0, LC0f), (LU, LUf)):
        nc.vector.tensor_copy(bf_t[:, :], f_t[:, :])

    b_i2s, b_s2s, b_c2c, b_up = ins["b_i2s"], ins["b_s2s"], ins["b_c2c"], ins["b_up"]
    for blk, (btile, bout) in ((0, (bi2s, bsg01)), (1, (bi2s_b, bsg23))):
        nc.sync.dma_start(out=btile[:, 0:1], in_=dv(b_i2s, 128 * blk, [[1, 128], [1, 1]]))
        nc.sync.dma_start(out=btile[:, 1:2], in_=dv(b_s2s, 128 * blk, [[1, 128], [1, 1]]))
        nc.vector.tensor_add(bout[:, :], btile[:, 0:1], btile[:, 1:2])
    nc.sync.dma_start(out=bc2c2[0:64, :], in_=dv(b_c2c, 0, [[1, 64], [1, 1]]))
    nc.sync.dma_start(out=bc2c2[64:128, :], in_=dv(b_c2c, 0, [[1, 64], [1, 1]]))
    nc.sync.dma_start(out=bup[:, :], in_=dv(b_up, 0, [[1, 128], [1, 1]]))

    # ---------------- input load ----------------
    # IN[c, b*4096 + p*64 + w] = inputs[b, c, p, w]
    IN = big.tile([64, B * H * W], BF, tag="IN")
    for b in range(B):
        nc.sync.dma_start(
            out=IN[:, b * H * W:(b + 1) * H * W],
            in_=dv(x_d, b * C * H * W, [[4096, 64], [1, 4096]]),
        )

    OUT = big.tile([128, B * H * W], F32, tag="OUT")
    IN_ap = IN[:, :]
    OUT_ap = OUT[:, :]

    def xprep(A2b, t):
        """Fill the x half (parts 64:128) of A2b for step t: x[c, 4p+b]
        for p in band(t), zero elsewhere."""
        xa = A2b[64:128, :]
        nc.gpsimd.memset(xa, 0.0)
        lo, hi = band(t)
        n = hi - lo + 1
        nc.gpsimd.tensor_copy(
            out=v(xa, 4 * lo, [[4, n], [1, 4]]),
            in_=v(IN_ap, 63 * lo + t, [[63, n], [4096, 4]]),
        )

    # ---------------- initial state ----------------
    A2 = st.tile([128, 256], BF, tag="A2", name="A2", bufs=3)
    nc.gpsimd.memset(A2[0:64, :], 0.0)
    xprep(A2, 0)
    # bf16 halves of c-state (matmul rhs; both re-based to partition 0 --
    # matmul rhs must share the lhsT's base partition)
    C2e = st.tile([64, 128], BF, tag="C2e", bufs=3)
    nc.gpsimd.memset(C2e[:, :], 0.0)
    C2o = st.tile([64, 128], BF, tag="C2o", bufs=3)
    nc.gpsimd.memset(C2o[:, :], 0.0)

    Uprev = None   # (U psum tile, t) pending upsample bias-add
    Hprev = None   # A2 tile holding h_{t-1} (rhs of this step)

    def emit_upsample(A2h, t):
        """Upsample matmul on the in-band part of h_t (held in A2h)."""
        U = ps.tile([128, 256], F32, tag="U")
        lo, hi = band(t)
        n = hi - lo + 1
        nc.tensor.matmul(
            U[:, 4 * lo:4 * (hi + 1)],
            LU[:, :],
            A2h[0:64, 4 * lo:4 * (hi + 1)],
            start=True, stop=True,
        )
        return U

    def emit_outadd(U, t):
        """OUT bias-add (DVE; GPSIMD cannot read PSUM).  Emitted after the
        h-muls so it executes in the DVE dead zone, and its U operand is
        ready early because the upsample runs right after the gate MMs."""
        lo, hi = band(t)
        n = hi - lo + 1
        nc.vector.tensor_scalar_add(
            v(OUT_ap, 63 * lo + t, [[4096, 4], [63, n]]),
            v(U[:, :], 4 * lo, [[1, 4], [4, n]]),
            bup[:, 0:1],
        )

    def store_block(p0, np_):
        """DMA OUT rows [p0, p0+np_) to DRAM (all images, all channels)."""
        for b in range(B):
            nc.sync.dma_start(
                out=dv(out_d, b * 128 * H * W + p0 * W, [[4096, 128], [1, np_ * W]]),
                in_=OUT[:, b * H * W + p0 * W: b * H * W + (p0 + np_) * W],
            )

    # ---------------- the recurrence ----------------
    for t in range(NW):
        # -- PE: c2c matmuls FIRST: their rhs (the bf16 c casts) lands
        #    ~600ns before h_{t-1} completes, so they fill the PE idle
        #    window ahead of the gate matmuls and Cp is ready well before
        #    STT_e needs it --
        Cp = ps.tile([128, 128], F32, tag="Cp")
        for w in (0, 1):
            cl = slice(64 * w, 64 * w + 64)
            nc.tensor.matmul(Cp[0:64, cl], LC1[:, :], C2e[:, cl],
                             start=True, stop=False, skip_group_check=True)
            nc.tensor.matmul(Cp[64:128, cl], LC1[:, :], C2o[:, cl],
                             start=True, stop=False, skip_group_check=True)
            # u'=1 out += Wc0 @ c-even (same kap)
            nc.tensor.matmul(Cp[64:128, cl], LC0[:, :], C2e[:, cl],
                             start=False, stop=True, skip_group_check=True)
            # u'=0 out += Wc0 @ c-odd at kap-1:
            if w == 0:
                # kap = 2j (j>=1) <- kap-1 = 2(j-1)+1: w=1 half, j-1
                nc.tensor.matmul(
                    v(Cp[0:64, :], 1, [[16, 4], [1, 15]]),
                    LC0[:, :], v(C2o[:, :], 64, [[16, 4], [1, 15]]),
                    start=False, stop=True, skip_group_check=True,
                )
            else:
                # kap = 2j+1 <- kap-1 = 2j: w=0 half, same j
                nc.tensor.matmul(
                    v(Cp[0:64, :], 64, [[16, 4], [1, 16]]),
                    LC0[:, :], v(C2o[:, :], 0, [[16, 4], [1, 16]]),
                    start=False, stop=True, skip_group_check=True,
                )

        # -- PE: gate matmuls (critical path) --
        P01 = ps.tile([128, 256], F32, tag="P01")
        P23 = ps.tile([128, 256], F32, tag="P23")
        for P, LA, LB in ((P01, LA01, LB01), (P23, LA23, LB23)):
            nc.tensor.matmul(P[:, :], LA[:, :], A2[:, :], start=True, stop=False)
            # Ws0 row-shift tap: out (b, p>=1) += Ws0 @ h[(b, p-1)]
            # (b-minor packing makes the row shift a flat column shift)
            nc.tensor.matmul(
                P[:, 4:256],
                LB[:, :],
                A2[0:64, 0:252],
                start=False, stop=True,
            )

        # -- PE: upsample of the previous step (off critical path) --
        if Hprev is not None:
            Uprev = (emit_upsample(Hprev, t - 1), t - 1)
            Hprev = None

        # -- ACT: the two sigmoid scatters P -> G (contiguous G halves) --
        G = tmp.tile([128, 512], BF, tag="G")
        Gap = G[:, :]
        for w, (P, bsg) in ((0, (P01, bsg01)), (1, (P23, bsg23))):
            nc.scalar.activation(
                v(Gap, 256 * w, [[64, 4], [16, 4], [1, 16]]),
                v(P[:, :], 0, [[64, 4], [1, 4], [4, 16]]),
                AF.Sigmoid, bias=bsg[:, 0:1],
            )

        # -- GpSimd: x for next step (off the critical chain) --
        A2n = st.tile([128, 256], BF, tag="A2", name="A2n", bufs=3)
        if t + 1 < NW:
            xprep(A2n, t + 1)

        # -- DVE gate math + ACT tanh, even stream first (runs in the
        #    shadow of sigmoid 2); all operands are contiguous slices --
        T1 = tmp.tile([128, 128], F32, tag="T1")
        T2 = tmp.tile([128, 128], F32, tag="T2")
        C2n = tmp.tile([128, 128], F32, tag="C2n")
        TH = tmp.tile([128, 128], BF, tag="TH")
        for w in (0, 1):
            g0 = 256 * w
            cl = slice(64 * w, 64 * w + 64)
            nc.vector.tensor_mul(T1[:, cl], G[:, g0:g0 + 64], G[:, g0 + 64:g0 + 128])
            nc.vector.scalar_tensor_tensor(
                out=T2[:, cl], in0=Cp[:, cl], scalar=bc2c2[:, 0:1],
                in1=G[:, g0 + 128:g0 + 192], op0=ALU.add, op1=ALU.mult,
            )
            nc.vector.tensor_add(C2n[:, cl], T1[:, cl], T2[:, cl])
            nc.scalar.activation(TH[:, cl], C2n[:, cl], AF.Tanh)
        # -- DVE: bf16 casts of c for the next c2c, emitted BEFORE the
        #    h-muls: they fill the DVE gap while tanh_o runs on ACT --
        C2en = st.tile([64, 128], BF, tag="C2e", bufs=3)
        nc.vector.tensor_copy(C2en[:, :], C2n[0:64, :])
        C2on = st.tile([64, 128], BF, tag="C2o", bufs=3)
        nc.vector.tensor_copy(C2on[:, :], C2n[64:128, :])
        # h_t = og * tanh(c_t) -> A2n[0:64], col 4*p2 + b, p2 = 4j + 2w + u
        for w in (0, 1):
            for u in (0, 1):
                nc.vector.tensor_mul(
                    v(A2n[0:64, :], 8 * w + 4 * u, [[16, 16], [1, 4]]),
                    v(G[64 * u:64 * u + 64, :], 256 * w + 192, [[1, 16], [16, 4]]),
                    v(TH[64 * u:64 * u + 64, :], 64 * w, [[1, 16], [16, 4]]),
                )

        if Uprev is not None:
            emit_outadd(*Uprev)
            Uprev = None

        if STAGGER_OUT and t >= 80 and (t - 80) % 16 == 0 and (t - 80) // 16 < 3:
            store_block(16 * ((t - 80) // 16), 16)

        A2 = A2n
        Hprev = A2n
        C2e = C2en
        C2o = C2on

    # ---------------- epilogue: last upsample + store ----------------
    U = emit_upsample(A2, NW - 1)
    emit_outadd(U, NW - 1)
    if STAGGER_OUT:
        store_block(48, 16)
    else:
        store_block(0, 64)


def build_nc():
    nc = bacc.Bacc("TRN2", target_bir_lowering=False, debug=False)
    ins = {
        "inputs": nc.dram_tensor("inputs", [B, C, H, W], BF, kind="ExternalInput").ap(),
        "w_i2s": nc.dram_tensor("w_i2s", [4 * HID, C], F32, kind="ExternalInput").ap(),
        "b_i2s": nc.dram_tensor("b_i2s", [4 * HID], F32, kind="ExternalInput").ap(),
        "w_s2s": nc.dram_tensor("w_s2s", [4 * HID, HID, 2], F32, kind="ExternalInput").ap(),
        "b_s2s": nc.dram_tensor("b_s2s", [4 * HID], F32, kind="ExternalInput").ap(),
        "w_c2c": nc.dram_tensor("w_c2c", [HID, HID, 2], F32, kind="ExternalInput").ap(),
        "b_c2c": nc.dram_tensor("b_c2c", [HID], F32, kind="ExternalInput").ap(),
        "w_up": nc.dram_tensor("w_up", [2 * HID, HID], F32, kind="ExternalInput").ap(),
        "b_up": nc.dram_tensor("b_up", [2 * HID], F32, kind="ExternalInput").ap(),
    }
    outs = {"out": nc.dram_tensor("out", [B, 2 * HID, H, W], F32, kind="ExternalOutput").ap()}
    with tile.TileContext(nc) as tc:
        with ExitStack() as ctx:
            build_kernel(ctx, tc, outs, ins)
    nc.compile()
    return nc


# ---------------------------------------------------------------------------
# Harness entry point: full inputs -> shard over 8 cores -> full output.
# ---------------------------------------------------------------------------
import ml_dtypes
from concourse.bass_utils import run_bass_kernel_spmd

N_CORES = 8
TRACE = False
LAST_EXEC_NS = None
LAST_RESULT = None
_NC = None


def _get_nc():
    global _NC
    if _NC is None:
        _NC = build_nc()
    return _NC


def kernel(**inputs):
    global LAST_EXEC_NS, LAST_RESULT
    nc = _get_nc()
    full = {k: np.ascontiguousarray(np.asarray(val, np.float32))
            for k, val in inputs.items()}
    xs = full["inputs"].astype(ml_dtypes.bfloat16)
    in_maps = []
    for i in range(N_CORES):
        m = dict(full)
        m["inputs"] = np.ascontiguousarray(xs[B * i:B * (i + 1)])
        in_maps.append(m)
    res = run_bass_kernel_spmd(nc, in_maps, list(range(N_CORES)), trace=TRACE)
    LAST_EXEC_NS = res.exec_time_ns
    LAST_RESULT = res
    return np.concatenate([res.results[i]["out"] for i in range(N_CORES)], axis=0)


# revision 23
# speedup vs baseline: 1.0838x; 1.0214x over previous
"""DiagonalLSTM Bass/Tile kernel for TRN2 (per-core shard: B=4 images).

Layout "DESIGN-E" (contiguous kap-parity split streams):
  State columns (A2 rhs / P gate tiles / U) are packed col = 4*p + b
  (b minor).  c-space tiles (Cp/T1/T2/C2n/TH and the G gate tile) are packed
  kap-parity-major: col = 64*w + 16*b + j where the c position is
  p2 = 2*kap + u, kap = 2*j + w, partition = 64*u + k.

  Why: sigmoid w (reading gate-tile P_w, which holds chans 128w:128w+128 =
  positions with kap%2 == w) then writes the CONTIGUOUS G half
  [256w, 256w+256), so the whole even-parity gate-math stream
  (T1/STT/add on DVE, tanh on ACT) runs on plain [128,64] slices in the
  shadow of sigmoid 2 / tanh_o, with no strided DVE penalty.  Only the odd
  tail (STT_o/add_o -> tanh_o -> h-mul_o) is serial between the sigmoids
  and the next step's gate matmuls.

  G[64u+k, 256w + 64q + 16b + j] = sigmoid(gate q at p2, chan k); the
  model's flat-split identity maps gate q of c-position (p2,k) to
  P_{kap%2} col 4*(16q + j) + b -- a 3-free-dim AP both sides.

Per step: 4 gate matmuls (K-packed [h;x], Ws0 row-shift via offset view),
8 half-width c2c matmuls (w=0 half first so STT_e can start early),
2 sigmoids, contiguous DVE gate math, 1 upsample matmul; the output
bias-add runs on Pool so it cannot block the DVE gate window.  Output is
stored in staggered 16-row blocks so the final DMA tail is 1/4 the image.
"""
from contextlib import ExitStack

import numpy as np

import concourse.bass as bass
import concourse.tile as tile
from concourse import bacc, mybir

F32 = mybir.dt.float32
BF = mybir.dt.bfloat16
AF = mybir.ActivationFunctionType
ALU = mybir.AluOpType

B = 4          # images per core
H = 64         # rows
W = 64         # cols
C = 64         # input channels
HID = 64       # hidden
NW = H + W - 1 # 127 diagonal steps

STAGGER_OUT = True


def v(ap, off, dims):
    """Custom view: keep ap's partition dim, replace free dims, add offset
    (in elements)."""
    return bass.AP(ap.tensor, ap.offset + off, [list(ap.ap[0])] + [list(d) for d in dims])


def dv(ap, off, dims):
    """Fully-custom view (DRAM side of DMAs): absolute offset, all dims."""
    return bass.AP(ap.tensor, off, [list(d) for d in dims])


def band(t):
    return max(0, t - (W - 1)), min(H - 1, t)


def build_kernel(ctx, tc, outs, ins):
    nc = tc.nc
    x_d = ins["inputs"]
    out_d = outs["out"]

    const = ctx.enter_context(tc.tile_pool(name="const", bufs=1))
    big = ctx.enter_context(tc.tile_pool(name="big", bufs=1))
    st = ctx.enter_context(tc.tile_pool(name="st", bufs=2))
    tmp = ctx.enter_context(tc.tile_pool(name="tmp", bufs=2))
    ps = ctx.enter_context(tc.tile_pool(name="ps", bufs=2, space="PSUM"))

    # ---------------- weights / biases (one-time prep) ----------------
    # lhsT layouts; matmul computes lhsT.T @ rhs.
    LA01 = const.tile([128, 128], BF, tag="LA01")  # [[Ws1 o=0:128].T ; [Wi2s o=0:128].T]
    LA23 = const.tile([128, 128], BF, tag="LA23")
    LB01 = const.tile([64, 128], BF, tag="LB01")   # Ws0[0:128].T
    LB23 = const.tile([64, 128], BF, tag="LB23")
    LC1 = const.tile([64, 64], BF, tag="LC1")      # Wc1.T
    LC0 = const.tile([64, 64], BF, tag="LC0")
    LU = const.tile([64, 128], BF, tag="LU")       # w_up.T
    LA01f = const.tile([128, 128], F32, tag="LA01f")
    LA23f = const.tile([128, 128], F32, tag="LA23f")
    LB01f = const.tile([64, 128], F32, tag="LB01f")
    LB23f = const.tile([64, 128], F32, tag="LB23f")
    LC1f = const.tile([64, 64], F32, tag="LC1f")
    LC0f = const.tile([64, 64], F32, tag="LC0f")
    LUf = const.tile([64, 128], F32, tag="LUf")
    bi2s = const.tile([128, 2], F32, tag="bi2s")    # col 0: b_i2s, col 1: b_s2s
    bsg01 = const.tile([128, 1], F32, tag="bsg01")
    bi2s_b = const.tile([128, 2], F32, tag="bi2s_b")
    bsg23 = const.tile([128, 1], F32, tag="bsg23")
    bc2c2 = const.tile([128, 1], F32, tag="bc2c2")
    bup = const.tile([128, 1], F32, tag="bup")

    w_s2s = ins["w_s2s"]   # [256, 64, 2] dram
    w_i2s = ins["w_i2s"]   # [256, 64]
    w_c2c = ins["w_c2c"]   # [64, 64, 2]
    w_up = ins["w_up"]     # [128, 64]

    for blk, LA, LB in ((0, LA01f, LB01f), (1, LA23f, LB23f)):
        # LA[kk,m] = Ws1[128*blk+m, kk] (kk<64) | Wi2s[128*blk+m, kk-64]
        nc.sync.dma_start(
            out=LA[0:64, :],
            in_=dv(w_s2s, 128 * blk * 128 + 1, [[2, 64], [128, 128]]),
        )
        nc.sync.dma_start(
            out=LA[64:128, :],
            in_=dv(w_i2s, 128 * blk * 64, [[1, 64], [64, 128]]),
        )
        nc.sync.dma_start(
            out=LB[:, :],
            in_=dv(w_s2s, 128 * blk * 128 + 0, [[2, 64], [128, 128]]),
        )
    nc.sync.dma_start(out=LC1f[:, :], in_=dv(w_c2c, 1, [[2, 64], [128, 64]]))
    nc.sync.dma_start(out=LC0f[:, :], in_=dv(w_c2c, 0, [[2, 64], [128, 64]]))
    nc.sync.dma_start(out=LUf[:, :], in_=dv(w_up, 0, [[1, 64], [64, 128]]))
    for bf_t, f_t in ((LA01, LA01f), (LA23, LA23f), (LB01, LB01f), (LB23, LB23f),
                      (LC1, LC1f), (LC0, LC0f), (LU, LUf)):
        nc.vector.tensor_copy(bf_t[:, :], f_t[:, :])

    b_i2s, b_s2s, b_c2c, b_up = ins["b_i2s"], ins["b_s2s"], ins["b_c2c"], ins["b_up"]
    for blk, (btile, bout) in ((0, (bi2s, bsg01)), (1, (bi2s_b, bsg23))):
        nc.sync.dma_start(out=btile[:, 0:1], in_=dv(b_i2s, 128 * blk, [[1, 128], [1, 1]]))
        nc.sync.dma_start(out=btile[:, 1:2], in_=dv(b_s2s, 128 * blk, [[1, 128], [1, 1]]))
        nc.vector.tensor_add(bout[:, :], btile[:, 0:1], btile[:, 1:2])
    nc.sync.dma_start(out=bc2c2[0:64, :], in_=dv(b_c2c, 0, [[1, 64], [1, 1]]))
    nc.sync.dma_start(out=bc2c2[64:128, :], in_=dv(b_c2c, 0, [[1, 64], [1, 1]]))
    nc.sync.dma_start(out=bup[:, :], in_=dv(b_up, 0, [[1, 128], [1, 1]]))

    # ---------------- input load ----------------
    # IN[c, b*4096 + p*64 + w] = inputs[b, c, p, w]
    IN = big.tile([64, B * H * W], BF, tag="IN")
    for b in range(B):
        nc.sync.dma_start(
            out=IN[:, b * H * W:(b + 1) * H * W],
            in_=dv(x_d, b * C * H * W, [[4096, 64], [1, 4096]]),
        )

    OUT = big.tile([128, B * H * W], F32, tag="OUT")
    IN_ap = IN[:, :]
    OUT_ap = OUT[:, :]

    def xprep(A2b, t):
        """Fill the x half (parts 64:128) of A2b for step t: x[c, 4p+b]
        for p in band(t), zero elsewhere."""
        xa = A2b[64:128, :]
        nc.gpsimd.memset(xa, 0.0)
        lo, hi = band(t)
        n = hi - lo + 1
        nc.gpsimd.tensor_copy(
            out=v(xa, 4 * lo, [[4, n], [1, 4]]),
            in_=v(IN_ap, 63 * lo + t, [[63, n], [4096, 4]]),
        )

    # ---------------- initial state ----------------
    A2 = st.tile([128, 256], BF, tag="A2", name="A2", bufs=3)
    nc.gpsimd.memset(A2[0:64, :], 0.0)
    xprep(A2, 0)
    # bf16 halves of c-state (matmul rhs; both re-based to partition 0 --
    # matmul rhs must share the lhsT's base partition)
    C2e = st.tile([64, 128], BF, tag="C2e", bufs=3)
    nc.gpsimd.memset(C2e[:, :], 0.0)
    C2o = st.tile([64, 128], BF, tag="C2o", bufs=3)
    nc.gpsimd.memset(C2o[:, :], 0.0)

    Uprev = None   # (U psum tile, t) pending upsample bias-add
    Hprev = None   # A2 tile holding h_{t-1} (rhs of this step)

    def emit_upsample(A2h, t):
        """Upsample matmul on the in-band part of h_t (held in A2h)."""
        U = ps.tile([128, 256], F32, tag="U")
        lo, hi = band(t)
        n = hi - lo + 1
        nc.tensor.matmul(
            U[:, 4 * lo:4 * (hi + 1)],
            LU[:, :],
            A2h[0:64, 4 * lo:4 * (hi + 1)],
            start=True, stop=True,
        )
        return U

    def emit_outadd(U, t):
        """OUT bias-add on the Scalar engine (Identity activation with a
        per-partition bias AP).  On DVE this op kept executing mid-window
        and blocked the gate math behind it in queue order; ACT has idle
        budget after the tanhs and can read PSUM directly."""
        lo, hi = band(t)
        n = hi - lo + 1
        nc.scalar.activation(
            v(OUT_ap, 63 * lo + t, [[4096, 4], [63, n]]),
            v(U[:, :], 4 * lo, [[1, 4], [4, n]]),
            AF.Identity, bias=bup[:, 0:1],
        )

    def store_block(p0, np_):
        """DMA OUT rows [p0, p0+np_) to DRAM (all images, all channels)."""
        for b in range(B):
            nc.sync.dma_start(
                out=dv(out_d, b * 128 * H * W + p0 * W, [[4096, 128], [1, np_ * W]]),
                in_=OUT[:, b * H * W + p0 * W: b * H * W + (p0 + np_) * W],
            )

    # ---------------- the recurrence ----------------
    for t in range(NW):
        # -- PE: c2c matmuls FIRST: their rhs (the bf16 c casts) lands
        #    ~600ns before h_{t-1} completes, so they fill the PE idle
        #    window ahead of the gate matmuls and Cp is ready well before
        #    STT_e needs it --
        Cp = ps.tile([128, 128], F32, tag="Cp")
        for w in (0, 1):
            cl = slice(64 * w, 64 * w + 64)
            nc.tensor.matmul(Cp[0:64, cl], LC1[:, :], C2e[:, cl],
                             start=True, stop=False, skip_group_check=True)
            nc.tensor.matmul(Cp[64:128, cl], LC1[:, :], C2o[:, cl],
                             start=True, stop=False, skip_group_check=True)
            # u'=1 out += Wc0 @ c-even (same kap)
            nc.tensor.matmul(Cp[64:128, cl], LC0[:, :], C2e[:, cl],
                             start=False, stop=True, skip_group_check=True)
            # u'=0 out += Wc0 @ c-odd at kap-1:
            if w == 0:
                # kap = 2j (j>=1) <- kap-1 = 2(j-1)+1: w=1 half, j-1
                nc.tensor.matmul(
                    v(Cp[0:64, :], 1, [[16, 4], [1, 15]]),
                    LC0[:, :], v(C2o[:, :], 64, [[16, 4], [1, 15]]),
                    start=False, stop=True, skip_group_check=True,
                )
            else:
                # kap = 2j+1 <- kap-1 = 2j: w=0 half, same j
                nc.tensor.matmul(
                    v(Cp[0:64, :], 64, [[16, 4], [1, 16]]),
                    LC0[:, :], v(C2o[:, :], 0, [[16, 4], [1, 16]]),
                    start=False, stop=True, skip_group_check=True,
                )

        # -- PE: gate matmuls (critical path) --
        P01 = ps.tile([128, 256], F32, tag="P01")
        P23 = ps.tile([128, 256], F32, tag="P23")
        for P, LA, LB in ((P01, LA01, LB01), (P23, LA23, LB23)):
            nc.tensor.matmul(P[:, :], LA[:, :], A2[:, :], start=True, stop=False)
            # Ws0 row-shift tap: out (b, p>=1) += Ws0 @ h[(b, p-1)]
            # (b-minor packing makes the row shift a flat column shift)
            nc.tensor.matmul(
                P[:, 4:256],
                LB[:, :],
                A2[0:64, 0:252],
                start=False, stop=True,
            )

        # -- PE: upsample of the previous step (off critical path) --
        if Hprev is not None:
            Uprev = (emit_upsample(Hprev, t - 1), t - 1)
            Hprev = None

        # -- ACT: the two sigmoid scatters P -> G (contiguous G halves) --
        G = tmp.tile([128, 512], BF, tag="G")
        Gap = G[:, :]
        for w, (P, bsg) in ((0, (P01, bsg01)), (1, (P23, bsg23))):
            nc.scalar.activation(
                v(Gap, 256 * w, [[64, 4], [16, 4], [1, 16]]),
                v(P[:, :], 0, [[64, 4], [1, 4], [4, 16]]),
                AF.Sigmoid, bias=bsg[:, 0:1],
            )

        # -- GpSimd: x for next step (off the critical chain) --
        A2n = st.tile([128, 256], BF, tag="A2", name="A2n", bufs=3)
        if t + 1 < NW:
            xprep(A2n, t + 1)

        # -- DVE gate math + ACT tanh, even stream first (runs in the
        #    shadow of sigmoid 2); all operands are contiguous slices --
        T1 = tmp.tile([128, 128], F32, tag="T1")
        T2 = tmp.tile([128, 128], F32, tag="T2")
        C2n = tmp.tile([128, 128], F32, tag="C2n")
        TH = tmp.tile([128, 128], BF, tag="TH")
        for w in (0, 1):
            g0 = 256 * w
            cl = slice(64 * w, 64 * w + 64)
            nc.vector.tensor_mul(T1[:, cl], G[:, g0:g0 + 64], G[:, g0 + 64:g0 + 128])
            nc.vector.scalar_tensor_tensor(
                out=T2[:, cl], in0=Cp[:, cl], scalar=bc2c2[:, 0:1],
                in1=G[:, g0 + 128:g0 + 192], op0=ALU.add, op1=ALU.mult,
            )
            nc.vector.tensor_add(C2n[:, cl], T1[:, cl], T2[:, cl])
            nc.scalar.activation(TH[:, cl], C2n[:, cl], AF.Tanh)
        # -- DVE: bf16 casts of c for the next c2c, emitted BEFORE the
        #    h-muls: they fill the DVE gap while tanh_o runs on ACT --
        C2en = st.tile([64, 128], BF, tag="C2e", bufs=3)
        nc.vector.tensor_copy(C2en[:, :], C2n[0:64, :])
        C2on = st.tile([64, 128], BF, tag="C2o", bufs=3)
        nc.vector.tensor_copy(C2on[:, :], C2n[64:128, :])
        # h_t = og * tanh(c_t) -> A2n[0:64], col 4*p2 + b, p2 = 4j + 2w + u
        for w in (0, 1):
            for u in (0, 1):
                nc.vector.tensor_mul(
                    v(A2n[0:64, :], 8 * w + 4 * u, [[16, 16], [1, 4]]),
                    v(G[64 * u:64 * u + 64, :], 256 * w + 192, [[1, 16], [16, 4]]),
                    v(TH[64 * u:64 * u + 64, :], 64 * w, [[1, 16], [16, 4]]),
                )

        if Uprev is not None:
            emit_outadd(*Uprev)
            Uprev = None

        if STAGGER_OUT and t >= 80 and (t - 80) % 16 == 0 and (t - 80) // 16 < 3:
            store_block(16 * ((t - 80) // 16), 16)

        A2 = A2n
        Hprev = A2n
        C2e = C2en
        C2o = C2on

    # ---------------- epilogue: last upsample + store ----------------
    U = emit_upsample(A2, NW - 1)
    emit_outadd(U, NW - 1)
    if STAGGER_OUT:
        store_block(48, 16)
    else:
        store_block(0, 64)


def build_nc():
    nc = bacc.Bacc("TRN2", target_bir_lowering=False, debug=False)
    ins = {
        "inputs": nc.dram_tensor("inputs", [B, C, H, W], BF, kind="ExternalInput").ap(),
        "w_i2s": nc.dram_tensor("w_i2s", [4 * HID, C], F32, kind="ExternalInput").ap(),
        "b_i2s": nc.dram_tensor("b_i2s", [4 * HID], F32, kind="ExternalInput").ap(),
        "w_s2s": nc.dram_tensor("w_s2s", [4 * HID, HID, 2], F32, kind="ExternalInput").ap(),
        "b_s2s": nc.dram_tensor("b_s2s", [4 * HID], F32, kind="ExternalInput").ap(),
        "w_c2c": nc.dram_tensor("w_c2c", [HID, HID, 2], F32, kind="ExternalInput").ap(),
        "b_c2c": nc.dram_tensor("b_c2c", [HID], F32, kind="ExternalInput").ap(),
        "w_up": nc.dram_tensor("w_up", [2 * HID, HID], F32, kind="ExternalInput").ap(),
        "b_up": nc.dram_tensor("b_up", [2 * HID], F32, kind="ExternalInput").ap(),
    }
    outs = {"out": nc.dram_tensor("out", [B, 2 * HID, H, W], F32, kind="ExternalOutput").ap()}
    with tile.TileContext(nc) as tc:
        with ExitStack() as ctx:
            build_kernel(ctx, tc, outs, ins)
    nc.compile()
    return nc


# ---------------------------------------------------------------------------
# Harness entry point: full inputs -> shard over 8 cores -> full output.
# ---------------------------------------------------------------------------
import ml_dtypes
from concourse.bass_utils import run_bass_kernel_spmd

N_CORES = 8
TRACE = False
LAST_EXEC_NS = None
LAST_RESULT = None
_NC = None


def _get_nc():
    global _NC
    if _NC is None:
        _NC = build_nc()
    return _NC


def kernel(**inputs):
    global LAST_EXEC_NS, LAST_RESULT
    nc = _get_nc()
    full = {k: np.ascontiguousarray(np.asarray(val, np.float32))
            for k, val in inputs.items()}
    xs = full["inputs"].astype(ml_dtypes.bfloat16)
    in_maps = []
    for i in range(N_CORES):
        m = dict(full)
        m["inputs"] = np.ascontiguousarray(xs[B * i:B * (i + 1)])
        in_maps.append(m)
    res = run_bass_kernel_spmd(nc, in_maps, list(range(N_CORES)), trace=TRACE)
    LAST_EXEC_NS = res.exec_time_ns
    LAST_RESULT = res
    return np.concatenate([res.results[i]["out"] for i in range(N_CORES)], axis=0)


# revision 27
# speedup vs baseline: 1.1063x; 1.0208x over previous
"""DiagonalLSTM Bass/Tile kernel for TRN2 (per-core shard: B=4 images).

Layout "DESIGN-E" (contiguous kap-parity split streams):
  State columns (A2 rhs / P gate tiles / U) are packed col = 4*p + b
  (b minor).  c-space tiles (Cp/T1/T2/C2n/TH and the G gate tile) are packed
  kap-parity-major: col = 64*w + 16*b + j where the c position is
  p2 = 2*kap + u, kap = 2*j + w, partition = 64*u + k.

  Why: sigmoid w (reading gate-tile P_w, which holds chans 128w:128w+128 =
  positions with kap%2 == w) then writes the CONTIGUOUS G half
  [256w, 256w+256), so the whole even-parity gate-math stream
  (T1/STT/add on DVE, tanh on ACT) runs on plain [128,64] slices in the
  shadow of sigmoid 2 / tanh_o, with no strided DVE penalty.  Only the odd
  tail (STT_o/add_o -> tanh_o -> h-mul_o) is serial between the sigmoids
  and the next step's gate matmuls.

  G[64u+k, 256w + 64q + 16b + j] = sigmoid(gate q at p2, chan k); the
  model's flat-split identity maps gate q of c-position (p2,k) to
  P_{kap%2} col 4*(16q + j) + b -- a 3-free-dim AP both sides.

Per step: 4 gate matmuls (K-packed [h;x], Ws0 row-shift via offset view),
8 half-width c2c matmuls (w=0 half first so STT_e can start early),
2 sigmoids, contiguous DVE gate math, 1 upsample matmul; the output
bias-add runs on Pool so it cannot block the DVE gate window.  Output is
stored in staggered 16-row blocks so the final DMA tail is 1/4 the image.
"""
from contextlib import ExitStack

import numpy as np

import concourse.bass as bass
import concourse.tile as tile
from concourse import bacc, mybir

F32 = mybir.dt.float32
BF = mybir.dt.bfloat16
AF = mybir.ActivationFunctionType
ALU = mybir.AluOpType

B = 4          # images per core
H = 64         # rows
W = 64         # cols
C = 64         # input channels
HID = 64       # hidden
NW = H + W - 1 # 127 diagonal steps

STAGGER_OUT = True


def v(ap, off, dims):
    """Custom view: keep ap's partition dim, replace free dims, add offset
    (in elements)."""
    return bass.AP(ap.tensor, ap.offset + off, [list(ap.ap[0])] + [list(d) for d in dims])


def dv(ap, off, dims):
    """Fully-custom view (DRAM side of DMAs): absolute offset, all dims."""
    return bass.AP(ap.tensor, off, [list(d) for d in dims])


def band(t):
    return max(0, t - (W - 1)), min(H - 1, t)


def build_kernel(ctx, tc, outs, ins):
    nc = tc.nc
    x_d = ins["inputs"]
    out_d = outs["out"]

    const = ctx.enter_context(tc.tile_pool(name="const", bufs=1))
    big = ctx.enter_context(tc.tile_pool(name="big", bufs=1))
    st = ctx.enter_context(tc.tile_pool(name="st", bufs=2))
    tmp = ctx.enter_context(tc.tile_pool(name="tmp", bufs=2))
    ps = ctx.enter_context(tc.tile_pool(name="ps", bufs=2, space="PSUM"))

    # ---------------- weights / biases (one-time prep) ----------------
    # lhsT layouts; matmul computes lhsT.T @ rhs.
    LA01 = const.tile([128, 128], BF, tag="LA01")  # [[Ws1 o=0:128].T ; [Wi2s o=0:128].T]
    LA23 = const.tile([128, 128], BF, tag="LA23")
    LB01 = const.tile([64, 128], BF, tag="LB01")   # Ws0[0:128].T
    LB23 = const.tile([64, 128], BF, tag="LB23")
    LC1 = const.tile([64, 64], BF, tag="LC1")      # Wc1.T
    LC0 = const.tile([64, 64], BF, tag="LC0")
    LU = const.tile([64, 128], BF, tag="LU")       # w_up.T
    LA01f = const.tile([128, 128], F32, tag="LA01f")
    LA23f = const.tile([128, 128], F32, tag="LA23f")
    LB01f = const.tile([64, 128], F32, tag="LB01f")
    LB23f = const.tile([64, 128], F32, tag="LB23f")
    LC1f = const.tile([64, 64], F32, tag="LC1f")
    LC0f = const.tile([64, 64], F32, tag="LC0f")
    LUf = const.tile([64, 128], F32, tag="LUf")
    bi2s = const.tile([128, 2], F32, tag="bi2s")    # col 0: b_i2s, col 1: b_s2s
    bsg01 = const.tile([128, 1], F32, tag="bsg01")
    bi2s_b = const.tile([128, 2], F32, tag="bi2s_b")
    bsg23 = const.tile([128, 1], F32, tag="bsg23")
    bc2c2 = const.tile([128, 1], F32, tag="bc2c2")
    bup = const.tile([128, 1], F32, tag="bup")

    w_s2s = ins["w_s2s"]   # [256, 64, 2] dram
    w_i2s = ins["w_i2s"]   # [256, 64]
    w_c2c = ins["w_c2c"]   # [64, 64, 2]
    w_up = ins["w_up"]     # [128, 64]

    for blk, LA, LB in ((0, LA01f, LB01f), (1, LA23f, LB23f)):
        # LA[kk,m] = Ws1[128*blk+m, kk] (kk<64) | Wi2s[128*blk+m, kk-64]
        nc.sync.dma_start(
            out=LA[0:64, :],
            in_=dv(w_s2s, 128 * blk * 128 + 1, [[2, 64], [128, 128]]),
        )
        nc.sync.dma_start(
            out=LA[64:128, :],
            in_=dv(w_i2s, 128 * blk * 64, [[1, 64], [64, 128]]),
        )
        nc.sync.dma_start(
            out=LB[:, :],
            in_=dv(w_s2s, 128 * blk * 128 + 0, [[2, 64], [128, 128]]),
        )
    nc.sync.dma_start(out=LC1f[:, :], in_=dv(w_c2c, 1, [[2, 64], [128, 64]]))
    nc.sync.dma_start(out=LC0f[:, :], in_=dv(w_c2c, 0, [[2, 64], [128, 64]]))
    nc.sync.dma_start(out=LUf[:, :], in_=dv(w_up, 0, [[1, 64], [64, 128]]))
    for bf_t, f_t in ((LA01, LA01f), (LA23, LA23f), (LB01, LB01f), (LB23, LB23f),
                      (LC1, LC1f), (LC0, LC0f), (LU, LUf)):
        nc.vector.tensor_copy(bf_t[:, :], f_t[:, :])

    b_i2s, b_s2s, b_c2c, b_up = ins["b_i2s"], ins["b_s2s"], ins["b_c2c"], ins["b_up"]
    for blk, (btile, bout) in ((0, (bi2s, bsg01)), (1, (bi2s_b, bsg23))):
        nc.sync.dma_start(out=btile[:, 0:1], in_=dv(b_i2s, 128 * blk, [[1, 128], [1, 1]]))
        nc.sync.dma_start(out=btile[:, 1:2], in_=dv(b_s2s, 128 * blk, [[1, 128], [1, 1]]))
        nc.vector.tensor_add(bout[:, :], btile[:, 0:1], btile[:, 1:2])
    nc.sync.dma_start(out=bc2c2[0:64, :], in_=dv(b_c2c, 0, [[1, 64], [1, 1]]))
    nc.sync.dma_start(out=bc2c2[64:128, :], in_=dv(b_c2c, 0, [[1, 64], [1, 1]]))
    nc.sync.dma_start(out=bup[:, :], in_=dv(b_up, 0, [[1, 128], [1, 1]]))

    # ---------------- input load ----------------
    # IN[c, b*4096 + p*64 + w] = inputs[b, c, p, w]
    IN = big.tile([64, B * H * W], BF, tag="IN")
    for b in range(B):
        nc.sync.dma_start(
            out=IN[:, b * H * W:(b + 1) * H * W],
            in_=dv(x_d, b * C * H * W, [[4096, 64], [1, 4096]]),
        )

    OUT = big.tile([128, B * H * W], F32, tag="OUT")
    IN_ap = IN[:, :]
    OUT_ap = OUT[:, :]

    SCR = big.tile([1, 4], BF, tag="SCR")

    def xprep(A2b, t, full_zero=True, dep=None):
        """Fill the x half (parts 64:128) of A2b for step t: x[c, 4p+b]
        for p in band(t), zero elsewhere.

        With the 3-deep A2 buffer rotation, A2b's x half holds x for step
        t-3, so only rows in band(t-3) \\ band(t) (at most 3, and only once
        t > 63) are stale -- the in-band copy overwrites the rest.

        The ~1us Pool gather must not overlap the DVE gate window (its
        SBUF traffic slowed concurrent DVE gate math ~3x), so `dep` pins
        it: a 1-element Pool read of h_{t-2} delays the copy until the
        previous step's h-muls are done -- the engine-idle dead zone."""
        xa = A2b[64:128, :]
        lo, hi = band(t)
        n = hi - lo + 1
        if dep is not None:
            nc.gpsimd.tensor_copy(out=SCR[:, 0:1], in_=dep[0:1, 0:1])
        if full_zero:
            nc.gpsimd.memset(xa, 0.0)
        else:
            lo3 = band(t - 3)[0]
            if lo > lo3:
                nc.gpsimd.memset(xa[:, 4 * lo3:4 * lo], 0.0)
        nc.gpsimd.tensor_copy(
            out=v(xa, 4 * lo, [[4, n], [1, 4]]),
            in_=v(IN_ap, 63 * lo + t, [[63, n], [4096, 4]]),
        )

    # ---------------- initial state ----------------
    A2 = st.tile([128, 256], BF, tag="A2", name="A2", bufs=3)
    nc.gpsimd.memset(A2[0:64, :], 0.0)
    xprep(A2, 0)
    # bf16 halves of c-state (matmul rhs; both re-based to partition 0 --
    # matmul rhs must share the lhsT's base partition)
    C2e = st.tile([64, 128], BF, tag="C2e", bufs=3)
    nc.gpsimd.memset(C2e[:, :], 0.0)
    C2o = st.tile([64, 128], BF, tag="C2o", bufs=3)
    nc.gpsimd.memset(C2o[:, :], 0.0)

    Uprev = None   # (U psum tile, t) pending upsample bias-add
    Hprev = None   # A2 tile holding h_{t-1} (rhs of this step)

    def emit_upsample(A2h, t):
        """Upsample matmul on the in-band part of h_t (held in A2h)."""
        U = ps.tile([128, 256], F32, tag="U")
        lo, hi = band(t)
        n = hi - lo + 1
        nc.tensor.matmul(
            U[:, 4 * lo:4 * (hi + 1)],
            LU[:, :],
            A2h[0:64, 4 * lo:4 * (hi + 1)],
            start=True, stop=True,
        )
        return U

    def emit_outadd(U, t):
        """OUT bias-add on the Scalar engine (Identity activation with a
        per-partition bias AP).  On DVE this op kept executing mid-window
        and blocked the gate math behind it in queue order; ACT has idle
        budget after the tanhs and can read PSUM directly."""
        lo, hi = band(t)
        n = hi - lo + 1
        nc.scalar.activation(
            v(OUT_ap, 63 * lo + t, [[4096, 4], [63, n]]),
            v(U[:, :], 4 * lo, [[1, 4], [4, n]]),
            AF.Identity, bias=bup[:, 0:1],
        )

    def store_block(p0, np_):
        """DMA OUT rows [p0, p0+np_) to DRAM (all images, all channels)."""
        for b in range(B):
            nc.sync.dma_start(
                out=dv(out_d, b * 128 * H * W + p0 * W, [[4096, 128], [1, np_ * W]]),
                in_=OUT[:, b * H * W + p0 * W: b * H * W + (p0 + np_) * W],
            )

    # ---------------- the recurrence ----------------
    for t in range(NW):
        # -- PE: c2c matmuls FIRST: their rhs (the bf16 c casts) lands
        #    ~600ns before h_{t-1} completes, so they fill the PE idle
        #    window ahead of the gate matmuls and Cp is ready well before
        #    STT_e needs it --
        Cp = ps.tile([128, 128], F32, tag="Cp")
        for w in (0, 1):
            cl = slice(64 * w, 64 * w + 64)
            nc.tensor.matmul(Cp[0:64, cl], LC1[:, :], C2e[:, cl],
                             start=True, stop=False, skip_group_check=True)
            nc.tensor.matmul(Cp[64:128, cl], LC1[:, :], C2o[:, cl],
                             start=True, stop=False, skip_group_check=True)
            # u'=1 out += Wc0 @ c-even (same kap)
            nc.tensor.matmul(Cp[64:128, cl], LC0[:, :], C2e[:, cl],
                             start=False, stop=True, skip_group_check=True)
            # u'=0 out += Wc0 @ c-odd at kap-1:
            if w == 0:
                # kap = 2j (j>=1) <- kap-1 = 2(j-1)+1: w=1 half, j-1
                nc.tensor.matmul(
                    v(Cp[0:64, :], 1, [[16, 4], [1, 15]]),
                    LC0[:, :], v(C2o[:, :], 64, [[16, 4], [1, 15]]),
                    start=False, stop=True, skip_group_check=True,
                )
            else:
                # kap = 2j+1 <- kap-1 = 2j: w=0 half, same j
                nc.tensor.matmul(
                    v(Cp[0:64, :], 64, [[16, 4], [1, 16]]),
                    LC0[:, :], v(C2o[:, :], 0, [[16, 4], [1, 16]]),
                    start=False, stop=True, skip_group_check=True,
                )

        # -- PE: gate matmuls (critical path) --
        P01 = ps.tile([128, 256], F32, tag="P01")
        P23 = ps.tile([128, 256], F32, tag="P23")
        for P, LA, LB in ((P01, LA01, LB01), (P23, LA23, LB23)):
            nc.tensor.matmul(P[:, :], LA[:, :], A2[:, :], start=True, stop=False)
            # Ws0 row-shift tap: out (b, p>=1) += Ws0 @ h[(b, p-1)]
            # (b-minor packing makes the row shift a flat column shift)
            nc.tensor.matmul(
                P[:, 4:256],
                LB[:, :],
                A2[0:64, 0:252],
                start=False, stop=True,
            )

        # -- PE: upsample of the previous step (off critical path) --
        if Hprev is not None:
            Uprev = (emit_upsample(Hprev, t - 1), t - 1)
            Hprev = None

        # -- ACT: the two sigmoid scatters P -> G (contiguous G halves) --
        G = tmp.tile([128, 512], BF, tag="G")
        Gap = G[:, :]
        for w, (P, bsg) in ((0, (P01, bsg01)), (1, (P23, bsg23))):
            nc.scalar.activation(
                v(Gap, 256 * w, [[64, 4], [16, 4], [1, 16]]),
                v(P[:, :], 0, [[64, 4], [1, 4], [4, 16]]),
                AF.Sigmoid, bias=bsg[:, 0:1],
            )

        # -- Pool: x for step t+1, pinned to the dead zone after the
        #    previous step's h-muls (dep on A2 = h_{t-1}) --
        A2n = st.tile([128, 256], BF, tag="A2", name="A2n", bufs=3)
        if t + 1 < NW:
            xprep(A2n, t + 1, full_zero=(t + 1 <= 2), dep=A2)

        # -- DVE gate math + ACT tanh, even stream first (runs in the
        #    shadow of sigmoid 2); all operands are contiguous slices --
        T1 = tmp.tile([128, 128], F32, tag="T1")
        T2 = tmp.tile([128, 128], F32, tag="T2")
        C2n = tmp.tile([128, 128], F32, tag="C2n")
        TH = tmp.tile([128, 128], BF, tag="TH")
        for w in (0, 1):
            g0 = 256 * w
            cl = slice(64 * w, 64 * w + 64)
            nc.vector.tensor_mul(T1[:, cl], G[:, g0:g0 + 64], G[:, g0 + 64:g0 + 128])
            nc.vector.scalar_tensor_tensor(
                out=T2[:, cl], in0=Cp[:, cl], scalar=bc2c2[:, 0:1],
                in1=G[:, g0 + 128:g0 + 192], op0=ALU.add, op1=ALU.mult,
            )
            nc.vector.tensor_add(C2n[:, cl], T1[:, cl], T2[:, cl])
            nc.scalar.activation(TH[:, cl], C2n[:, cl], AF.Tanh)
        # -- DVE: bf16 casts of c for the next c2c, emitted BEFORE the
        #    h-muls: they fill the DVE gap while tanh_o runs on ACT --
        C2en = st.tile([64, 128], BF, tag="C2e", bufs=3)
        nc.vector.tensor_copy(C2en[:, :], C2n[0:64, :])
        C2on = st.tile([64, 128], BF, tag="C2o", bufs=3)
        nc.vector.tensor_copy(C2on[:, :], C2n[64:128, :])
        # h_t = og * tanh(c_t) -> A2n[0:64], col 4*p2 + b, p2 = 4j + 2w + u
        for w in (0, 1):
            for u in (0, 1):
                nc.vector.tensor_mul(
                    v(A2n[0:64, :], 8 * w + 4 * u, [[16, 16], [1, 4]]),
                    v(G[64 * u:64 * u + 64, :], 256 * w + 192, [[1, 16], [16, 4]]),
                    v(TH[64 * u:64 * u + 64, :], 64 * w, [[1, 16], [16, 4]]),
                )

        if Uprev is not None:
            emit_outadd(*Uprev)
            Uprev = None

        if STAGGER_OUT and t >= 80 and (t - 80) % 16 == 0 and (t - 80) // 16 < 3:
            store_block(16 * ((t - 80) // 16), 16)

        A2 = A2n
        Hprev = A2n
        C2e = C2en
        C2o = C2on

    # ---------------- epilogue: last upsample + store ----------------
    U = emit_upsample(A2, NW - 1)
    emit_outadd(U, NW - 1)
    if STAGGER_OUT:
        store_block(48, 16)
    else:
        store_block(0, 64)


def build_nc():
    nc = bacc.Bacc("TRN2", target_bir_lowering=False, debug=False)
    ins = {
        "inputs": nc.dram_tensor("inputs", [B, C, H, W], BF, kind="ExternalInput").ap(),
        "w_i2s": nc.dram_tensor("w_i2s", [4 * HID, C], F32, kind="ExternalInput").ap(),
        "b_i2s": nc.dram_tensor("b_i2s", [4 * HID], F32, kind="ExternalInput").ap(),
        "w_s2s": nc.dram_tensor("w_s2s", [4 * HID, HID, 2], F32, kind="ExternalInput").ap(),
        "b_s2s": nc.dram_tensor("b_s2s", [4 * HID], F32, kind="ExternalInput").ap(),
        "w_c2c": nc.dram_tensor("w_c2c", [HID, HID, 2], F32, kind="ExternalInput").ap(),
        "b_c2c": nc.dram_tensor("b_c2c", [HID], F32, kind="ExternalInput").ap(),
        "w_up": nc.dram_tensor("w_up", [2 * HID, HID], F32, kind="ExternalInput").ap(),
        "b_up": nc.dram_tensor("b_up", [2 * HID], F32, kind="ExternalInput").ap(),
    }
    outs = {"out": nc.dram_tensor("out", [B, 2 * HID, H, W], F32, kind="ExternalOutput").ap()}
    with tile.TileContext(nc) as tc:
        with ExitStack() as ctx:
            build_kernel(ctx, tc, outs, ins)
    nc.compile()
    return nc


# ---------------------------------------------------------------------------
# Harness entry point: full inputs -> shard over 8 cores -> full output.
# ---------------------------------------------------------------------------
import ml_dtypes
from concourse.bass_utils import run_bass_kernel_spmd

N_CORES = 8
TRACE = False
LAST_EXEC_NS = None
LAST_RESULT = None
_NC = None


def _get_nc():
    global _NC
    if _NC is None:
        _NC = build_nc()
    return _NC


def kernel(**inputs):
    global LAST_EXEC_NS, LAST_RESULT
    nc = _get_nc()
    full = {k: np.ascontiguousarray(np.asarray(val, np.float32))
            for k, val in inputs.items()}
    xs = full["inputs"].astype(ml_dtypes.bfloat16)
    in_maps = []
    for i in range(N_CORES):
        m = dict(full)
        m["inputs"] = np.ascontiguousarray(xs[B * i:B * (i + 1)])
        in_maps.append(m)
    res = run_bass_kernel_spmd(nc, in_maps, list(range(N_CORES)), trace=TRACE)
    LAST_EXEC_NS = res.exec_time_ns
    LAST_RESULT = res
    return np.concatenate([res.results[i]["out"] for i in range(N_CORES)], axis=0)


# revision 30
# speedup vs baseline: 1.1518x; 1.0411x over previous
"""DiagonalLSTM Bass/Tile kernel for TRN2 (per-core shard: B=4 images).

Layout "DESIGN-E" (contiguous kap-parity split streams):
  State columns (A2 rhs / P gate tiles / U) are packed col = 4*p + b
  (b minor).  c-space tiles (Cp/T1/T2/C2n/TH and the G gate tile) are packed
  kap-parity-major: col = 64*w + 16*b + j where the c position is
  p2 = 2*kap + u, kap = 2*j + w, partition = 64*u + k.

  Why: sigmoid w (reading gate-tile P_w, which holds chans 128w:128w+128 =
  positions with kap%2 == w) then writes the CONTIGUOUS G half
  [256w, 256w+256), so the whole even-parity gate-math stream
  (T1/STT/add on DVE, tanh on ACT) runs on plain [128,64] slices in the
  shadow of sigmoid 2 / tanh_o, with no strided DVE penalty.  Only the odd
  tail (STT_o/add_o -> tanh_o -> h-mul_o) is serial between the sigmoids
  and the next step's gate matmuls.

  G[64u+k, 256w + 64q + 16b + j] = sigmoid(gate q at p2, chan k); the
  model's flat-split identity maps gate q of c-position (p2,k) to
  P_{kap%2} col 4*(16q + j) + b -- a 3-free-dim AP both sides.

Per step: 4 gate matmuls (K-packed [h;x], Ws0 row-shift via offset view),
8 half-width c2c matmuls (w=0 half first so STT_e can start early),
2 sigmoids, contiguous DVE gate math, 1 upsample matmul; the output
bias-add runs on Pool so it cannot block the DVE gate window.  Output is
stored in staggered 16-row blocks so the final DMA tail is 1/4 the image.
"""
from contextlib import ExitStack

import numpy as np

import concourse.bass as bass
import concourse.tile as tile
from concourse import bacc, mybir

F32 = mybir.dt.float32
BF = mybir.dt.bfloat16
AF = mybir.ActivationFunctionType
ALU = mybir.AluOpType

B = 4          # images per core
H = 64         # rows
W = 64         # cols
C = 64         # input channels
HID = 64       # hidden
NW = H + W - 1 # 127 diagonal steps

STAGGER_OUT = True


def v(ap, off, dims):
    """Custom view: keep ap's partition dim, replace free dims, add offset
    (in elements)."""
    return bass.AP(ap.tensor, ap.offset + off, [list(ap.ap[0])] + [list(d) for d in dims])


def dv(ap, off, dims):
    """Fully-custom view (DRAM side of DMAs): absolute offset, all dims."""
    return bass.AP(ap.tensor, off, [list(d) for d in dims])


def band(t):
    return max(0, t - (W - 1)), min(H - 1, t)


def build_kernel(ctx, tc, outs, ins):
    nc = tc.nc
    x_d = ins["inputs"]
    out_d = outs["out"]

    const = ctx.enter_context(tc.tile_pool(name="const", bufs=1))
    big = ctx.enter_context(tc.tile_pool(name="big", bufs=1))
    st = ctx.enter_context(tc.tile_pool(name="st", bufs=2))
    tmp = ctx.enter_context(tc.tile_pool(name="tmp", bufs=2))
    ps = ctx.enter_context(tc.tile_pool(name="ps", bufs=2, space="PSUM"))

    # ---------------- weights / biases (one-time prep) ----------------
    # lhsT layouts; matmul computes lhsT.T @ rhs.
    LA01 = const.tile([128, 128], BF, tag="LA01")  # [[Ws1 o=0:128].T ; [Wi2s o=0:128].T]
    LA23 = const.tile([128, 128], BF, tag="LA23")
    LB01 = const.tile([64, 128], BF, tag="LB01")   # Ws0[0:128].T
    LB23 = const.tile([64, 128], BF, tag="LB23")
    LC1 = const.tile([64, 64], BF, tag="LC1")      # Wc1.T
    LC0 = const.tile([64, 64], BF, tag="LC0")
    LU = const.tile([64, 128], BF, tag="LU")       # w_up.T
    LA01f = const.tile([128, 128], F32, tag="LA01f")
    LA23f = const.tile([128, 128], F32, tag="LA23f")
    LB01f = const.tile([64, 128], F32, tag="LB01f")
    LB23f = const.tile([64, 128], F32, tag="LB23f")
    LC1f = const.tile([64, 64], F32, tag="LC1f")
    LC0f = const.tile([64, 64], F32, tag="LC0f")
    LUf = const.tile([64, 128], F32, tag="LUf")
    bi2s = const.tile([128, 2], F32, tag="bi2s")    # col 0: b_i2s, col 1: b_s2s
    bsg01 = const.tile([128, 1], F32, tag="bsg01")
    bi2s_b = const.tile([128, 2], F32, tag="bi2s_b")
    bsg23 = const.tile([128, 1], F32, tag="bsg23")
    bc2c2 = const.tile([128, 1], F32, tag="bc2c2")
    bup = const.tile([128, 1], F32, tag="bup")

    w_s2s = ins["w_s2s"]   # [256, 64, 2] dram
    w_i2s = ins["w_i2s"]   # [256, 64]
    w_c2c = ins["w_c2c"]   # [64, 64, 2]
    w_up = ins["w_up"]     # [128, 64]

    for blk, LA, LB in ((0, LA01f, LB01f), (1, LA23f, LB23f)):
        # LA[kk,m] = Ws1[128*blk+m, kk] (kk<64) | Wi2s[128*blk+m, kk-64]
        nc.sync.dma_start(
            out=LA[0:64, :],
            in_=dv(w_s2s, 128 * blk * 128 + 1, [[2, 64], [128, 128]]),
        )
        nc.sync.dma_start(
            out=LA[64:128, :],
            in_=dv(w_i2s, 128 * blk * 64, [[1, 64], [64, 128]]),
        )
        nc.sync.dma_start(
            out=LB[:, :],
            in_=dv(w_s2s, 128 * blk * 128 + 0, [[2, 64], [128, 128]]),
        )
    nc.sync.dma_start(out=LC1f[:, :], in_=dv(w_c2c, 1, [[2, 64], [128, 64]]))
    nc.sync.dma_start(out=LC0f[:, :], in_=dv(w_c2c, 0, [[2, 64], [128, 64]]))
    nc.sync.dma_start(out=LUf[:, :], in_=dv(w_up, 0, [[1, 64], [64, 128]]))
    for bf_t, f_t in ((LA01, LA01f), (LA23, LA23f), (LB01, LB01f), (LB23, LB23f),
                      (LC1, LC1f), (LC0, LC0f), (LU, LUf)):
        nc.vector.tensor_copy(bf_t[:, :], f_t[:, :])

    b_i2s, b_s2s, b_c2c, b_up = ins["b_i2s"], ins["b_s2s"], ins["b_c2c"], ins["b_up"]
    for blk, (btile, bout) in ((0, (bi2s, bsg01)), (1, (bi2s_b, bsg23))):
        nc.sync.dma_start(out=btile[:, 0:1], in_=dv(b_i2s, 128 * blk, [[1, 128], [1, 1]]))
        nc.sync.dma_start(out=btile[:, 1:2], in_=dv(b_s2s, 128 * blk, [[1, 128], [1, 1]]))
        nc.vector.tensor_add(bout[:, :], btile[:, 0:1], btile[:, 1:2])
    nc.sync.dma_start(out=bc2c2[0:64, :], in_=dv(b_c2c, 0, [[1, 64], [1, 1]]))
    nc.sync.dma_start(out=bc2c2[64:128, :], in_=dv(b_c2c, 0, [[1, 64], [1, 1]]))
    nc.sync.dma_start(out=bup[:, :], in_=dv(b_up, 0, [[1, 128], [1, 1]]))

    # ---------------- input load ----------------
    # IN[c, b*4096 + p*64 + w] = inputs[b, c, p, w]
    IN = big.tile([64, B * H * W], BF, tag="IN")
    for b in range(B):
        nc.sync.dma_start(
            out=IN[:, b * H * W:(b + 1) * H * W],
            in_=dv(x_d, b * C * H * W, [[4096, 64], [1, 4096]]),
        )

    OUT = big.tile([128, B * H * W], F32, tag="OUT")
    IN_ap = IN[:, :]
    OUT_ap = OUT[:, :]

    SCR = big.tile([1, 4], BF, tag="SCR")

    def xprep(A2b, t, full_zero=True, dep=None):
        """Fill the x half (parts 64:128) of A2b for step t: x[c, 4p+b]
        for p in band(t), zero elsewhere.

        With the 3-deep A2 buffer rotation, A2b's x half holds x for step
        t-3, so only rows in band(t-3) \\ band(t) (at most 3, and only once
        t > 63) are stale -- the in-band copy overwrites the rest.

        The ~1us Pool gather must not overlap the DVE gate window (its
        SBUF traffic slowed concurrent DVE gate math ~3x), so `dep` pins
        it: a 1-element Pool read of h_{t-2} delays the copy until the
        previous step's h-muls are done -- the engine-idle dead zone."""
        xa = A2b[64:128, :]
        lo, hi = band(t)
        n = hi - lo + 1
        if dep is not None:
            nc.gpsimd.tensor_copy(out=SCR[:, 0:1], in_=dep[0:1, 12:13])
        if full_zero:
            nc.gpsimd.memset(xa, 0.0)
        else:
            lo4 = band(t - 4)[0]
            if lo > lo4:
                nc.gpsimd.memset(xa[:, 4 * lo4:4 * lo], 0.0)
        nc.gpsimd.tensor_copy(
            out=v(xa, 4 * lo, [[4, n], [1, 4]]),
            in_=v(IN_ap, 63 * lo + t, [[63, n], [4096, 4]]),
        )

    # ---------------- initial state ----------------
    # Four persistent A2 buffers used round-robin (not pool-rotated: the
    # same tensor identity lets xprep skip re-zeroing cols that an earlier
    # pass of the same buffer already zeroed, and avoids pool-slot FIFO
    # serialization).
    A2T = [big.tile([128, 256], BF, tag=f"A2buf{i}", name=f"A2buf{i}")
           for i in range(4)]
    A2 = A2T[0]
    nc.gpsimd.memset(A2[0:64, :], 0.0)
    xprep(A2, 0)
    # bf16 halves of c-state (matmul rhs; both re-based to partition 0 --
    # matmul rhs must share the lhsT's base partition)
    C2e = st.tile([64, 128], BF, tag="C2e", bufs=3)
    nc.gpsimd.memset(C2e[:, :], 0.0)
    C2o = st.tile([64, 128], BF, tag="C2o", bufs=3)
    nc.gpsimd.memset(C2o[:, :], 0.0)

    Uprev = None   # (U psum tile, t) pending upsample bias-add
    Hprev = None   # A2 tile holding h_{t-1} (rhs of this step)

    def emit_upsample(A2h, t):
        """Upsample matmul on the in-band part of h_t (held in A2h)."""
        U = ps.tile([128, 256], F32, tag="U")
        lo, hi = band(t)
        n = hi - lo + 1
        nc.tensor.matmul(
            U[:, 4 * lo:4 * (hi + 1)],
            LU[:, :],
            A2h[0:64, 4 * lo:4 * (hi + 1)],
            start=True, stop=True,
        )
        return U

    def emit_outadd(U, t):
        """OUT bias-add on the Scalar engine (Identity activation with a
        per-partition bias AP).  On DVE this op kept executing mid-window
        and blocked the gate math behind it in queue order; ACT has idle
        budget after the tanhs and can read PSUM directly."""
        lo, hi = band(t)
        n = hi - lo + 1
        nc.scalar.activation(
            v(OUT_ap, 63 * lo + t, [[4096, 4], [63, n]]),
            v(U[:, :], 4 * lo, [[1, 4], [4, n]]),
            AF.Identity, bias=bup[:, 0:1],
        )

    def store_block(p0, np_):
        """DMA OUT rows [p0, p0+np_) to DRAM (all images, all channels)."""
        for b in range(B):
            nc.sync.dma_start(
                out=dv(out_d, b * 128 * H * W + p0 * W, [[4096, 128], [1, np_ * W]]),
                in_=OUT[:, b * H * W + p0 * W: b * H * W + (p0 + np_) * W],
            )

    # ---------------- the recurrence ----------------
    for t in range(NW):
        # -- PE: c2c matmuls FIRST: their rhs (the bf16 c casts) lands
        #    ~600ns before h_{t-1} completes, so they fill the PE idle
        #    window ahead of the gate matmuls and Cp is ready well before
        #    STT_e needs it --
        Cp = ps.tile([128, 128], F32, tag="Cp")
        for w in (0, 1):
            cl = slice(64 * w, 64 * w + 64)
            nc.tensor.matmul(Cp[0:64, cl], LC1[:, :], C2e[:, cl],
                             start=True, stop=False, skip_group_check=True)
            nc.tensor.matmul(Cp[64:128, cl], LC1[:, :], C2o[:, cl],
                             start=True, stop=False, skip_group_check=True)
            # u'=1 out += Wc0 @ c-even (same kap)
            nc.tensor.matmul(Cp[64:128, cl], LC0[:, :], C2e[:, cl],
                             start=False, stop=True, skip_group_check=True)
            # u'=0 out += Wc0 @ c-odd at kap-1:
            if w == 0:
                # kap = 2j (j>=1) <- kap-1 = 2(j-1)+1: w=1 half, j-1
                nc.tensor.matmul(
                    v(Cp[0:64, :], 1, [[16, 4], [1, 15]]),
                    LC0[:, :], v(C2o[:, :], 64, [[16, 4], [1, 15]]),
                    start=False, stop=True, skip_group_check=True,
                )
            else:
                # kap = 2j+1 <- kap-1 = 2j: w=0 half, same j
                nc.tensor.matmul(
                    v(Cp[0:64, :], 64, [[16, 4], [1, 16]]),
                    LC0[:, :], v(C2o[:, :], 0, [[16, 4], [1, 16]]),
                    start=False, stop=True, skip_group_check=True,
                )

        # -- PE: gate matmuls (critical path) --
        P01 = ps.tile([128, 256], F32, tag="P01")
        P23 = ps.tile([128, 256], F32, tag="P23")
        for P, LA, LB in ((P01, LA01, LB01), (P23, LA23, LB23)):
            nc.tensor.matmul(P[:, :], LA[:, :], A2[:, :], start=True, stop=False)
            # Ws0 row-shift tap: out (b, p>=1) += Ws0 @ h[(b, p-1)]
            # (b-minor packing makes the row shift a flat column shift)
            nc.tensor.matmul(
                P[:, 4:256],
                LB[:, :],
                A2[0:64, 0:252],
                start=False, stop=True,
            )

        # -- PE: upsample of the previous step (off critical path) --
        if Hprev is not None:
            Uprev = (emit_upsample(Hprev, t - 1), t - 1)
            Hprev = None

        # -- ACT: the two sigmoid scatters P -> G (contiguous G halves) --
        G = tmp.tile([128, 512], BF, tag="G")
        Gap = G[:, :]
        for w, (P, bsg) in ((0, (P01, bsg01)), (1, (P23, bsg23))):
            nc.scalar.activation(
                v(Gap, 256 * w, [[64, 4], [16, 4], [1, 16]]),
                v(P[:, :], 0, [[64, 4], [1, 4], [4, 16]]),
                AF.Sigmoid, bias=bsg[:, 0:1],
            )

        # -- Pool: x for step t+1, pinned to the dead zone after the
        #    previous step's h-muls (dep on A2 = h_{t-1}) --
        A2n = A2T[(t + 1) % 4]
        if t + 1 < NW:
            xprep(A2n, t + 1, full_zero=(t + 1 <= 3), dep=A2)

        # -- DVE gate math + ACT tanh, even stream first (runs in the
        #    shadow of sigmoid 2); all operands are contiguous slices --
        T1 = tmp.tile([128, 128], F32, tag="T1")
        T2 = tmp.tile([128, 128], F32, tag="T2")
        C2n = tmp.tile([128, 128], F32, tag="C2n")
        TH = tmp.tile([128, 128], BF, tag="TH")
        for w in (0, 1):
            g0 = 256 * w
            cl = slice(64 * w, 64 * w + 64)
            nc.vector.tensor_mul(T1[:, cl], G[:, g0:g0 + 64], G[:, g0 + 64:g0 + 128])
            nc.vector.scalar_tensor_tensor(
                out=T2[:, cl], in0=Cp[:, cl], scalar=bc2c2[:, 0:1],
                in1=G[:, g0 + 128:g0 + 192], op0=ALU.add, op1=ALU.mult,
            )
            nc.vector.tensor_add(C2n[:, cl], T1[:, cl], T2[:, cl])
            nc.scalar.activation(TH[:, cl], C2n[:, cl], AF.Tanh)
        # -- DVE: bf16 casts of c for the next c2c, emitted BEFORE the
        #    h-muls: they fill the DVE gap while tanh_o runs on ACT --
        C2en = st.tile([64, 128], BF, tag="C2e", bufs=3)
        nc.vector.tensor_copy(C2en[:, :], C2n[0:64, :])
        C2on = st.tile([64, 128], BF, tag="C2o", bufs=3)
        nc.vector.tensor_copy(C2on[:, :], C2n[64:128, :])
        # h_t = og * tanh(c_t) -> A2n[0:64], col 4*p2 + b, p2 = 4j + 2w + u
        for w in (0, 1):
            for u in (0, 1):
                nc.vector.tensor_mul(
                    v(A2n[0:64, :], 8 * w + 4 * u, [[16, 16], [1, 4]]),
                    v(G[64 * u:64 * u + 64, :], 256 * w + 192, [[1, 16], [16, 4]]),
                    v(TH[64 * u:64 * u + 64, :], 64 * w, [[1, 16], [16, 4]]),
                )

        if Uprev is not None:
            emit_outadd(*Uprev)
            Uprev = None

        if STAGGER_OUT and t >= 80 and (t - 80) % 16 == 0 and (t - 80) // 16 < 3:
            store_block(16 * ((t - 80) // 16), 16)

        A2 = A2n
        Hprev = A2n
        C2e = C2en
        C2o = C2on

    # ---------------- epilogue: last upsample + store ----------------
    U = emit_upsample(A2, NW - 1)
    emit_outadd(U, NW - 1)
    if STAGGER_OUT:
        store_block(48, 16)
    else:
        store_block(0, 64)


def build_nc():
    nc = bacc.Bacc("TRN2", target_bir_lowering=False, debug=False)
    ins = {
        "inputs": nc.dram_tensor("inputs", [B, C, H, W], BF, kind="ExternalInput").ap(),
        "w_i2s": nc.dram_tensor("w_i2s", [4 * HID, C], F32, kind="ExternalInput").ap(),
        "b_i2s": nc.dram_tensor("b_i2s", [4 * HID], F32, kind="ExternalInput").ap(),
        "w_s2s": nc.dram_tensor("w_s2s", [4 * HID, HID, 2], F32, kind="ExternalInput").ap(),
        "b_s2s": nc.dram_tensor("b_s2s", [4 * HID], F32, kind="ExternalInput").ap(),
        "w_c2c": nc.dram_tensor("w_c2c", [HID, HID, 2], F32, kind="ExternalInput").ap(),
        "b_c2c": nc.dram_tensor("b_c2c", [HID], F32, kind="ExternalInput").ap(),
        "w_up": nc.dram_tensor("w_up", [2 * HID, HID], F32, kind="ExternalInput").ap(),
        "b_up": nc.dram_tensor("b_up", [2 * HID], F32, kind="ExternalInput").ap(),
    }
    outs = {"out": nc.dram_tensor("out", [B, 2 * HID, H, W], F32, kind="ExternalOutput").ap()}
    with tile.TileContext(nc) as tc:
        with ExitStack() as ctx:
            build_kernel(ctx, tc, outs, ins)
    nc.compile()
    return nc


# ---------------------------------------------------------------------------
# Harness entry point: full inputs -> shard over 8 cores -> full output.
# ---------------------------------------------------------------------------
import ml_dtypes
from concourse.bass_utils import run_bass_kernel_spmd

N_CORES = 8
TRACE = False
LAST_EXEC_NS = None
LAST_RESULT = None
_NC = None


def _get_nc():
    global _NC
    if _NC is None:
        _NC = build_nc()
    return _NC


def kernel(**inputs):
    global LAST_EXEC_NS, LAST_RESULT
    nc = _get_nc()
    full = {k: np.ascontiguousarray(np.asarray(val, np.float32))
            for k, val in inputs.items()}
    xs = full["inputs"].astype(ml_dtypes.bfloat16)
    in_maps = []
    for i in range(N_CORES):
        m = dict(full)
        m["inputs"] = np.ascontiguousarray(xs[B * i:B * (i + 1)])
        in_maps.append(m)
    res = run_bass_kernel_spmd(nc, in_maps, list(range(N_CORES)), trace=TRACE)
    LAST_EXEC_NS = res.exec_time_ns
    LAST_RESULT = res
    return np.concatenate([res.results[i]["out"] for i in range(N_CORES)], axis=0)


# revision 31
# speedup vs baseline: 1.1525x; 1.0006x over previous
"""DiagonalLSTM Bass/Tile kernel for TRN2 (per-core shard: B=4 images).

Layout "DESIGN-E" (contiguous kap-parity split streams):
  State columns (A2 rhs / P gate tiles / U) are packed col = 4*p + b
  (b minor).  c-space tiles (Cp/T1/T2/C2n/TH and the G gate tile) are packed
  kap-parity-major: col = 64*w + 16*b + j where the c position is
  p2 = 2*kap + u, kap = 2*j + w, partition = 64*u + k.

  Why: sigmoid w (reading gate-tile P_w, which holds chans 128w:128w+128 =
  positions with kap%2 == w) then writes the CONTIGUOUS G half
  [256w, 256w+256), so the whole even-parity gate-math stream
  (T1/STT/add on DVE, tanh on ACT) runs on plain [128,64] slices in the
  shadow of sigmoid 2 / tanh_o, with no strided DVE penalty.  Only the odd
  tail (STT_o/add_o -> tanh_o -> h-mul_o) is serial between the sigmoids
  and the next step's gate matmuls.

  G[64u+k, 256w + 64q + 16b + j] = sigmoid(gate q at p2, chan k); the
  model's flat-split identity maps gate q of c-position (p2,k) to
  P_{kap%2} col 4*(16q + j) + b -- a 3-free-dim AP both sides.

Per step: 4 gate matmuls (K-packed [h;x], Ws0 row-shift via offset view),
8 half-width c2c matmuls (w=0 half first so STT_e can start early),
2 sigmoids, contiguous DVE gate math, 1 upsample matmul; the output
bias-add runs on Pool so it cannot block the DVE gate window.  Output is
stored in staggered 16-row blocks so the final DMA tail is 1/4 the image.
"""
from contextlib import ExitStack

import numpy as np

import concourse.bass as bass
import concourse.tile as tile
from concourse import bacc, mybir

F32 = mybir.dt.float32
BF = mybir.dt.bfloat16
AF = mybir.ActivationFunctionType
ALU = mybir.AluOpType

B = 4          # images per core
H = 64         # rows
W = 64         # cols
C = 64         # input channels
HID = 64       # hidden
NW = H + W - 1 # 127 diagonal steps

STAGGER_OUT = True


def v(ap, off, dims):
    """Custom view: keep ap's partition dim, replace free dims, add offset
    (in elements)."""
    return bass.AP(ap.tensor, ap.offset + off, [list(ap.ap[0])] + [list(d) for d in dims])


def dv(ap, off, dims):
    """Fully-custom view (DRAM side of DMAs): absolute offset, all dims."""
    return bass.AP(ap.tensor, off, [list(d) for d in dims])


def band(t):
    return max(0, t - (W - 1)), min(H - 1, t)


def build_kernel(ctx, tc, outs, ins):
    nc = tc.nc
    x_d = ins["inputs"]
    out_d = outs["out"]

    const = ctx.enter_context(tc.tile_pool(name="const", bufs=1))
    big = ctx.enter_context(tc.tile_pool(name="big", bufs=1))
    st = ctx.enter_context(tc.tile_pool(name="st", bufs=2))
    tmp = ctx.enter_context(tc.tile_pool(name="tmp", bufs=2))
    ps = ctx.enter_context(tc.tile_pool(name="ps", bufs=2, space="PSUM"))

    # ---------------- weights / biases (one-time prep) ----------------
    # lhsT layouts; matmul computes lhsT.T @ rhs.
    LA01 = const.tile([128, 128], BF, tag="LA01")  # [[Ws1 o=0:128].T ; [Wi2s o=0:128].T]
    LA23 = const.tile([128, 128], BF, tag="LA23")
    LB01 = const.tile([64, 128], BF, tag="LB01")   # Ws0[0:128].T
    LB23 = const.tile([64, 128], BF, tag="LB23")
    LC1 = const.tile([64, 64], BF, tag="LC1")      # Wc1.T
    LC0 = const.tile([64, 64], BF, tag="LC0")
    LU = const.tile([64, 128], BF, tag="LU")       # w_up.T
    LA01f = const.tile([128, 128], F32, tag="LA01f")
    LA23f = const.tile([128, 128], F32, tag="LA23f")
    LB01f = const.tile([64, 128], F32, tag="LB01f")
    LB23f = const.tile([64, 128], F32, tag="LB23f")
    LC1f = const.tile([64, 64], F32, tag="LC1f")
    LC0f = const.tile([64, 64], F32, tag="LC0f")
    LUf = const.tile([64, 128], F32, tag="LUf")
    bi2s = const.tile([128, 2], F32, tag="bi2s")    # col 0: b_i2s, col 1: b_s2s
    bsg01 = const.tile([128, 1], F32, tag="bsg01")
    bi2s_b = const.tile([128, 2], F32, tag="bi2s_b")
    bsg23 = const.tile([128, 1], F32, tag="bsg23")
    bc2c2 = const.tile([128, 1], F32, tag="bc2c2")
    bup = const.tile([128, 1], F32, tag="bup")

    w_s2s = ins["w_s2s"]   # [256, 64, 2] dram
    w_i2s = ins["w_i2s"]   # [256, 64]
    w_c2c = ins["w_c2c"]   # [64, 64, 2]
    w_up = ins["w_up"]     # [128, 64]

    for blk, LA, LB in ((0, LA01f, LB01f), (1, LA23f, LB23f)):
        # LA[kk,m] = Ws1[128*blk+m, kk] (kk<64) | Wi2s[128*blk+m, kk-64]
        nc.sync.dma_start(
            out=LA[0:64, :],
            in_=dv(w_s2s, 128 * blk * 128 + 1, [[2, 64], [128, 128]]),
        )
        nc.sync.dma_start(
            out=LA[64:128, :],
            in_=dv(w_i2s, 128 * blk * 64, [[1, 64], [64, 128]]),
        )
        nc.sync.dma_start(
            out=LB[:, :],
            in_=dv(w_s2s, 128 * blk * 128 + 0, [[2, 64], [128, 128]]),
        )
    nc.sync.dma_start(out=LC1f[:, :], in_=dv(w_c2c, 1, [[2, 64], [128, 64]]))
    nc.sync.dma_start(out=LC0f[:, :], in_=dv(w_c2c, 0, [[2, 64], [128, 64]]))
    nc.sync.dma_start(out=LUf[:, :], in_=dv(w_up, 0, [[1, 64], [64, 128]]))
    for bf_t, f_t in ((LA01, LA01f), (LA23, LA23f), (LB01, LB01f), (LB23, LB23f),
                      (LC1, LC1f), (LC0, LC0f), (LU, LUf)):
        nc.vector.tensor_copy(bf_t[:, :], f_t[:, :])

    b_i2s, b_s2s, b_c2c, b_up = ins["b_i2s"], ins["b_s2s"], ins["b_c2c"], ins["b_up"]
    for blk, (btile, bout) in ((0, (bi2s, bsg01)), (1, (bi2s_b, bsg23))):
        nc.sync.dma_start(out=btile[:, 0:1], in_=dv(b_i2s, 128 * blk, [[1, 128], [1, 1]]))
        nc.sync.dma_start(out=btile[:, 1:2], in_=dv(b_s2s, 128 * blk, [[1, 128], [1, 1]]))
        nc.vector.tensor_add(bout[:, :], btile[:, 0:1], btile[:, 1:2])
    nc.sync.dma_start(out=bc2c2[0:64, :], in_=dv(b_c2c, 0, [[1, 64], [1, 1]]))
    nc.sync.dma_start(out=bc2c2[64:128, :], in_=dv(b_c2c, 0, [[1, 64], [1, 1]]))
    nc.sync.dma_start(out=bup[:, :], in_=dv(b_up, 0, [[1, 128], [1, 1]]))

    # ---------------- input load ----------------
    # IN[c, b*4096 + p*64 + w] = inputs[b, c, p, w]
    IN = big.tile([64, B * H * W], BF, tag="IN")
    for b in range(B):
        nc.sync.dma_start(
            out=IN[:, b * H * W:(b + 1) * H * W],
            in_=dv(x_d, b * C * H * W, [[4096, 64], [1, 4096]]),
        )

    OUT = big.tile([128, B * H * W], F32, tag="OUT")
    IN_ap = IN[:, :]
    OUT_ap = OUT[:, :]

    SCR = big.tile([1, 4], BF, tag="SCR")

    def xprep(A2b, t, full_zero=True, dep=None):
        """Fill the x half (parts 64:128) of A2b for step t: x[c, 4p+b]
        for p in band(t), zero elsewhere.

        With the 3-deep A2 buffer rotation, A2b's x half holds x for step
        t-3, so only rows in band(t-3) \\ band(t) (at most 3, and only once
        t > 63) are stale -- the in-band copy overwrites the rest.

        The ~1us Pool gather must not overlap the DVE gate window (its
        SBUF traffic slowed concurrent DVE gate math ~3x), so `dep` pins
        it: a 1-element Pool read of h_{t-2} delays the copy until the
        previous step's h-muls are done -- the engine-idle dead zone."""
        xa = A2b[64:128, :]
        lo, hi = band(t)
        n = hi - lo + 1
        if dep is not None:
            nc.gpsimd.tensor_copy(out=SCR[:, 0:1], in_=dep[0:1, 12:13])
        if full_zero:
            nc.gpsimd.memset(xa, 0.0)
        else:
            lo4 = band(t - 4)[0]
            if lo > lo4:
                nc.gpsimd.memset(xa[:, 4 * lo4:4 * lo], 0.0)
        nc.gpsimd.tensor_copy(
            out=v(xa, 4 * lo, [[4, n], [1, 4]]),
            in_=v(IN_ap, 63 * lo + t, [[63, n], [4096, 4]]),
        )

    # ---------------- initial state ----------------
    # Four persistent A2 buffers used round-robin (not pool-rotated: the
    # same tensor identity lets xprep skip re-zeroing cols that an earlier
    # pass of the same buffer already zeroed, and avoids pool-slot FIFO
    # serialization).
    A2T = [big.tile([128, 256], BF, tag=f"A2buf{i}", name=f"A2buf{i}")
           for i in range(4)]
    A2 = A2T[0]
    nc.gpsimd.memset(A2[0:64, :], 0.0)
    xprep(A2, 0)
    # bf16 halves of c-state (matmul rhs; both re-based to partition 0 --
    # matmul rhs must share the lhsT's base partition)
    C2e = st.tile([64, 128], BF, tag="C2e", bufs=3)
    nc.gpsimd.memset(C2e[:, :], 0.0)
    C2o = st.tile([64, 128], BF, tag="C2o", bufs=3)
    nc.gpsimd.memset(C2o[:, :], 0.0)

    Uprev = None   # (U psum tile, t) pending upsample bias-add
    Hprev = None   # A2 tile holding h_{t-1} (rhs of this step)

    def emit_upsample(A2h, t):
        """Upsample matmul on the in-band part of h_t (held in A2h)."""
        U = ps.tile([128, 256], F32, tag="U")
        lo, hi = band(t)
        n = hi - lo + 1
        nc.tensor.matmul(
            U[:, 4 * lo:4 * (hi + 1)],
            LU[:, :],
            A2h[0:64, 4 * lo:4 * (hi + 1)],
            start=True, stop=True,
        )
        return U

    def emit_outadd(U, t):
        """OUT bias-add on the Scalar engine (Identity activation with a
        per-partition bias AP).  On DVE this op kept executing mid-window
        and blocked the gate math behind it in queue order; ACT has idle
        budget after the tanhs and can read PSUM directly."""
        lo, hi = band(t)
        n = hi - lo + 1
        nc.scalar.activation(
            v(OUT_ap, 63 * lo + t, [[4096, 4], [63, n]]),
            v(U[:, :], 4 * lo, [[1, 4], [4, n]]),
            AF.Identity, bias=bup[:, 0:1],
        )

    def store_block(p0, np_):
        """DMA OUT rows [p0, p0+np_) to DRAM (all images, all channels)."""
        for b in range(B):
            nc.sync.dma_start(
                out=dv(out_d, b * 128 * H * W + p0 * W, [[4096, 128], [1, np_ * W]]),
                in_=OUT[:, b * H * W + p0 * W: b * H * W + (p0 + np_) * W],
            )

    # ---------------- the recurrence ----------------
    for t in range(NW):
        # -- Pool: x for step t+1, pinned to the dead zone after the
        #    previous step's h-muls (tiny dep read of h_{t-1}); emitted
        #    FIRST so its WAR semaphore threshold is recorded against the
        #    previous iteration's PE counter, not this one's --
        A2n = A2T[(t + 1) % 4]
        if t + 1 < NW:
            xprep(A2n, t + 1, full_zero=(t + 1 <= 3), dep=A2)

        # -- PE: c2c matmuls FIRST: their rhs (the bf16 c casts) lands
        #    ~600ns before h_{t-1} completes, so they fill the PE idle
        #    window ahead of the gate matmuls and Cp is ready well before
        #    STT_e needs it --
        Cp = ps.tile([128, 128], F32, tag="Cp")
        for w in (0, 1):
            cl = slice(64 * w, 64 * w + 64)
            nc.tensor.matmul(Cp[0:64, cl], LC1[:, :], C2e[:, cl],
                             start=True, stop=False, skip_group_check=True)
            nc.tensor.matmul(Cp[64:128, cl], LC1[:, :], C2o[:, cl],
                             start=True, stop=False, skip_group_check=True)
            # u'=1 out += Wc0 @ c-even (same kap)
            nc.tensor.matmul(Cp[64:128, cl], LC0[:, :], C2e[:, cl],
                             start=False, stop=True, skip_group_check=True)
            # u'=0 out += Wc0 @ c-odd at kap-1:
            if w == 0:
                # kap = 2j (j>=1) <- kap-1 = 2(j-1)+1: w=1 half, j-1
                nc.tensor.matmul(
                    v(Cp[0:64, :], 1, [[16, 4], [1, 15]]),
                    LC0[:, :], v(C2o[:, :], 64, [[16, 4], [1, 15]]),
                    start=False, stop=True, skip_group_check=True,
                )
            else:
                # kap = 2j+1 <- kap-1 = 2j: w=0 half, same j
                nc.tensor.matmul(
                    v(Cp[0:64, :], 64, [[16, 4], [1, 16]]),
                    LC0[:, :], v(C2o[:, :], 0, [[16, 4], [1, 16]]),
                    start=False, stop=True, skip_group_check=True,
                )

        # -- PE: gate matmuls (critical path) --
        P01 = ps.tile([128, 256], F32, tag="P01")
        P23 = ps.tile([128, 256], F32, tag="P23")
        for P, LA, LB in ((P01, LA01, LB01), (P23, LA23, LB23)):
            nc.tensor.matmul(P[:, :], LA[:, :], A2[:, :], start=True, stop=False)
            # Ws0 row-shift tap: out (b, p>=1) += Ws0 @ h[(b, p-1)]
            # (b-minor packing makes the row shift a flat column shift)
            nc.tensor.matmul(
                P[:, 4:256],
                LB[:, :],
                A2[0:64, 0:252],
                start=False, stop=True,
            )

        # -- PE: upsample of the previous step (off critical path) --
        if Hprev is not None:
            Uprev = (emit_upsample(Hprev, t - 1), t - 1)
            Hprev = None

        # -- ACT: the two sigmoid scatters P -> G (contiguous G halves) --
        G = tmp.tile([128, 512], BF, tag="G")
        Gap = G[:, :]
        for w, (P, bsg) in ((0, (P01, bsg01)), (1, (P23, bsg23))):
            nc.scalar.activation(
                v(Gap, 256 * w, [[64, 4], [16, 4], [1, 16]]),
                v(P[:, :], 0, [[64, 4], [1, 4], [4, 16]]),
                AF.Sigmoid, bias=bsg[:, 0:1],
            )


        # -- DVE gate math + ACT tanh, even stream first (runs in the
        #    shadow of sigmoid 2); all operands are contiguous slices --
        T1 = tmp.tile([128, 128], F32, tag="T1")
        T2 = tmp.tile([128, 128], F32, tag="T2")
        C2n = tmp.tile([128, 128], F32, tag="C2n")
        TH = tmp.tile([128, 128], BF, tag="TH")
        for w in (0, 1):
            g0 = 256 * w
            cl = slice(64 * w, 64 * w + 64)
            nc.vector.tensor_mul(T1[:, cl], G[:, g0:g0 + 64], G[:, g0 + 64:g0 + 128])
            nc.vector.scalar_tensor_tensor(
                out=T2[:, cl], in0=Cp[:, cl], scalar=bc2c2[:, 0:1],
                in1=G[:, g0 + 128:g0 + 192], op0=ALU.add, op1=ALU.mult,
            )
            nc.vector.tensor_add(C2n[:, cl], T1[:, cl], T2[:, cl])
            nc.scalar.activation(TH[:, cl], C2n[:, cl], AF.Tanh)
        # -- DVE: bf16 casts of c for the next c2c, emitted BEFORE the
        #    h-muls: they fill the DVE gap while tanh_o runs on ACT --
        C2en = st.tile([64, 128], BF, tag="C2e", bufs=3)
        nc.vector.tensor_copy(C2en[:, :], C2n[0:64, :])
        C2on = st.tile([64, 128], BF, tag="C2o", bufs=3)
        nc.vector.tensor_copy(C2on[:, :], C2n[64:128, :])
        # h_t = og * tanh(c_t) -> A2n[0:64], col 4*p2 + b, p2 = 4j + 2w + u
        for w in (0, 1):
            for u in (0, 1):
                nc.vector.tensor_mul(
                    v(A2n[0:64, :], 8 * w + 4 * u, [[16, 16], [1, 4]]),
                    v(G[64 * u:64 * u + 64, :], 256 * w + 192, [[1, 16], [16, 4]]),
                    v(TH[64 * u:64 * u + 64, :], 64 * w, [[1, 16], [16, 4]]),
                )

        if Uprev is not None:
            emit_outadd(*Uprev)
            Uprev = None

        if STAGGER_OUT and t >= 80 and (t - 80) % 16 == 0 and (t - 80) // 16 < 3:
            store_block(16 * ((t - 80) // 16), 16)

        A2 = A2n
        Hprev = A2n
        C2e = C2en
        C2o = C2on

    # ---------------- epilogue: last upsample + store ----------------
    U = emit_upsample(A2, NW - 1)
    emit_outadd(U, NW - 1)
    if STAGGER_OUT:
        store_block(48, 16)
    else:
        store_block(0, 64)


def build_nc():
    nc = bacc.Bacc("TRN2", target_bir_lowering=False, debug=False)
    ins = {
        "inputs": nc.dram_tensor("inputs", [B, C, H, W], BF, kind="ExternalInput").ap(),
        "w_i2s": nc.dram_tensor("w_i2s", [4 * HID, C], F32, kind="ExternalInput").ap(),
        "b_i2s": nc.dram_tensor("b_i2s", [4 * HID], F32, kind="ExternalInput").ap(),
        "w_s2s": nc.dram_tensor("w_s2s", [4 * HID, HID, 2], F32, kind="ExternalInput").ap(),
        "b_s2s": nc.dram_tensor("b_s2s", [4 * HID], F32, kind="ExternalInput").ap(),
        "w_c2c": nc.dram_tensor("w_c2c", [HID, HID, 2], F32, kind="ExternalInput").ap(),
        "b_c2c": nc.dram_tensor("b_c2c", [HID], F32, kind="ExternalInput").ap(),
        "w_up": nc.dram_tensor("w_up", [2 * HID, HID], F32, kind="ExternalInput").ap(),
        "b_up": nc.dram_tensor("b_up", [2 * HID], F32, kind="ExternalInput").ap(),
    }
    outs = {"out": nc.dram_tensor("out", [B, 2 * HID, H, W], F32, kind="ExternalOutput").ap()}
    with tile.TileContext(nc) as tc:
        with ExitStack() as ctx:
            build_kernel(ctx, tc, outs, ins)
    nc.compile()
    return nc


# ---------------------------------------------------------------------------
# Harness entry point: full inputs -> shard over 8 cores -> full output.
# ---------------------------------------------------------------------------
import ml_dtypes
from concourse.bass_utils import run_bass_kernel_spmd

N_CORES = 8
TRACE = False
LAST_EXEC_NS = None
LAST_RESULT = None
_NC = None


def _get_nc():
    global _NC
    if _NC is None:
        _NC = build_nc()
    return _NC


def kernel(**inputs):
    global LAST_EXEC_NS, LAST_RESULT
    nc = _get_nc()
    full = {k: np.ascontiguousarray(np.asarray(val, np.float32))
            for k, val in inputs.items()}
    xs = full["inputs"].astype(ml_dtypes.bfloat16)
    in_maps = []
    for i in range(N_CORES):
        m = dict(full)
        m["inputs"] = np.ascontiguousarray(xs[B * i:B * (i + 1)])
        in_maps.append(m)
    res = run_bass_kernel_spmd(nc, in_maps, list(range(N_CORES)), trace=TRACE)
    LAST_EXEC_NS = res.exec_time_ns
    LAST_RESULT = res
    return np.concatenate([res.results[i]["out"] for i in range(N_CORES)], axis=0)


# revision 32
# speedup vs baseline: 1.1592x; 1.0058x over previous
"""DiagonalLSTM Bass/Tile kernel for TRN2 (per-core shard: B=4 images).

Layout "DESIGN-E" (contiguous kap-parity split streams):
  State columns (A2 rhs / P gate tiles / U) are packed col = 4*p + b
  (b minor).  c-space tiles (Cp/T1/T2/C2n/TH and the G gate tile) are packed
  kap-parity-major: col = 64*w + 16*b + j where the c position is
  p2 = 2*kap + u, kap = 2*j + w, partition = 64*u + k.

  Why: sigmoid w (reading gate-tile P_w, which holds chans 128w:128w+128 =
  positions with kap%2 == w) then writes the CONTIGUOUS G half
  [256w, 256w+256), so the whole even-parity gate-math stream
  (T1/STT/add on DVE, tanh on ACT) runs on plain [128,64] slices in the
  shadow of sigmoid 2 / tanh_o, with no strided DVE penalty.  Only the odd
  tail (STT_o/add_o -> tanh_o -> h-mul_o) is serial between the sigmoids
  and the next step's gate matmuls.

  G[64u+k, 256w + 64q + 16b + j] = sigmoid(gate q at p2, chan k); the
  model's flat-split identity maps gate q of c-position (p2,k) to
  P_{kap%2} col 4*(16q + j) + b -- a 3-free-dim AP both sides.

Per step: 4 gate matmuls (K-packed [h;x], Ws0 row-shift via offset view),
8 half-width c2c matmuls (w=0 half first so STT_e can start early),
2 sigmoids, contiguous DVE gate math, 1 upsample matmul; the output
bias-add runs on Pool so it cannot block the DVE gate window.  Output is
stored in staggered 16-row blocks so the final DMA tail is 1/4 the image.
"""
from contextlib import ExitStack

import numpy as np

import concourse.bass as bass
import concourse.tile as tile
from concourse import bacc, mybir

F32 = mybir.dt.float32
BF = mybir.dt.bfloat16
AF = mybir.ActivationFunctionType
ALU = mybir.AluOpType

B = 4          # images per core
H = 64         # rows
W = 64         # cols
C = 64         # input channels
HID = 64       # hidden
NW = H + W - 1 # 127 diagonal steps

STAGGER_OUT = True


def v(ap, off, dims):
    """Custom view: keep ap's partition dim, replace free dims, add offset
    (in elements)."""
    return bass.AP(ap.tensor, ap.offset + off, [list(ap.ap[0])] + [list(d) for d in dims])


def dv(ap, off, dims):
    """Fully-custom view (DRAM side of DMAs): absolute offset, all dims."""
    return bass.AP(ap.tensor, off, [list(d) for d in dims])


def band(t):
    return max(0, t - (W - 1)), min(H - 1, t)


def build_kernel(ctx, tc, outs, ins):
    nc = tc.nc
    x_d = ins["inputs"]
    out_d = outs["out"]

    const = ctx.enter_context(tc.tile_pool(name="const", bufs=1))
    big = ctx.enter_context(tc.tile_pool(name="big", bufs=1))
    st = ctx.enter_context(tc.tile_pool(name="st", bufs=2))
    tmp = ctx.enter_context(tc.tile_pool(name="tmp", bufs=2))
    ps = ctx.enter_context(tc.tile_pool(name="ps", bufs=2, space="PSUM"))

    # ---------------- weights / biases (one-time prep) ----------------
    # lhsT layouts; matmul computes lhsT.T @ rhs.
    LA01 = const.tile([128, 128], BF, tag="LA01")  # [[Ws1 o=0:128].T ; [Wi2s o=0:128].T]
    LA23 = const.tile([128, 128], BF, tag="LA23")
    LB01 = const.tile([64, 128], BF, tag="LB01")   # Ws0[0:128].T
    LB23 = const.tile([64, 128], BF, tag="LB23")
    LC1 = const.tile([64, 64], BF, tag="LC1")      # Wc1.T
    LC0 = const.tile([64, 64], BF, tag="LC0")
    LU = const.tile([64, 128], BF, tag="LU")       # w_up.T
    LA01f = const.tile([128, 128], F32, tag="LA01f")
    LA23f = const.tile([128, 128], F32, tag="LA23f")
    LB01f = const.tile([64, 128], F32, tag="LB01f")
    LB23f = const.tile([64, 128], F32, tag="LB23f")
    LC1f = const.tile([64, 64], F32, tag="LC1f")
    LC0f = const.tile([64, 64], F32, tag="LC0f")
    LUf = const.tile([64, 128], F32, tag="LUf")
    bi2s = const.tile([128, 2], F32, tag="bi2s")    # col 0: b_i2s, col 1: b_s2s
    bsg01 = const.tile([128, 1], F32, tag="bsg01")
    bi2s_b = const.tile([128, 2], F32, tag="bi2s_b")
    bsg23 = const.tile([128, 1], F32, tag="bsg23")
    bc2c2 = const.tile([128, 1], F32, tag="bc2c2")
    bup = const.tile([128, 1], F32, tag="bup")

    w_s2s = ins["w_s2s"]   # [256, 64, 2] dram
    w_i2s = ins["w_i2s"]   # [256, 64]
    w_c2c = ins["w_c2c"]   # [64, 64, 2]
    w_up = ins["w_up"]     # [128, 64]

    for blk, LA, LB in ((0, LA01f, LB01f), (1, LA23f, LB23f)):
        # LA[kk,m] = Ws1[128*blk+m, kk] (kk<64) | Wi2s[128*blk+m, kk-64]
        nc.sync.dma_start(
            out=LA[0:64, :],
            in_=dv(w_s2s, 128 * blk * 128 + 1, [[2, 64], [128, 128]]),
        )
        nc.sync.dma_start(
            out=LA[64:128, :],
            in_=dv(w_i2s, 128 * blk * 64, [[1, 64], [64, 128]]),
        )
        nc.sync.dma_start(
            out=LB[:, :],
            in_=dv(w_s2s, 128 * blk * 128 + 0, [[2, 64], [128, 128]]),
        )
    nc.sync.dma_start(out=LC1f[:, :], in_=dv(w_c2c, 1, [[2, 64], [128, 64]]))
    nc.sync.dma_start(out=LC0f[:, :], in_=dv(w_c2c, 0, [[2, 64], [128, 64]]))
    nc.sync.dma_start(out=LUf[:, :], in_=dv(w_up, 0, [[1, 64], [64, 128]]))
    for bf_t, f_t in ((LA01, LA01f), (LA23, LA23f), (LB01, LB01f), (LB23, LB23f),
                      (LC1, LC1f), (LC0, LC0f), (LU, LUf)):
        nc.vector.tensor_copy(bf_t[:, :], f_t[:, :])

    b_i2s, b_s2s, b_c2c, b_up = ins["b_i2s"], ins["b_s2s"], ins["b_c2c"], ins["b_up"]
    for blk, (btile, bout) in ((0, (bi2s, bsg01)), (1, (bi2s_b, bsg23))):
        nc.sync.dma_start(out=btile[:, 0:1], in_=dv(b_i2s, 128 * blk, [[1, 128], [1, 1]]))
        nc.sync.dma_start(out=btile[:, 1:2], in_=dv(b_s2s, 128 * blk, [[1, 128], [1, 1]]))
        nc.vector.tensor_add(bout[:, :], btile[:, 0:1], btile[:, 1:2])
    nc.sync.dma_start(out=bc2c2[0:64, :], in_=dv(b_c2c, 0, [[1, 64], [1, 1]]))
    nc.sync.dma_start(out=bc2c2[64:128, :], in_=dv(b_c2c, 0, [[1, 64], [1, 1]]))
    nc.sync.dma_start(out=bup[:, :], in_=dv(b_up, 0, [[1, 128], [1, 1]]))

    # ---------------- input load ----------------
    # IN[c, b*4096 + p*64 + w] = inputs[b, c, p, w]
    IN = big.tile([64, B * H * W], BF, tag="IN")
    for b in range(B):
        nc.sync.dma_start(
            out=IN[:, b * H * W:(b + 1) * H * W],
            in_=dv(x_d, b * C * H * W, [[4096, 64], [1, 4096]]),
        )

    OUT = big.tile([128, B * H * W], F32, tag="OUT")
    IN_ap = IN[:, :]
    OUT_ap = OUT[:, :]

    SCR = big.tile([1, 4], BF, tag="SCR")

    def xprep(A2b, t, full_zero=True, dep=None):
        """Fill the x half (parts 64:128) of A2b for step t: x[c, 4p+b]
        for p in band(t), zero elsewhere.

        With the 3-deep A2 buffer rotation, A2b's x half holds x for step
        t-3, so only rows in band(t-3) \\ band(t) (at most 3, and only once
        t > 63) are stale -- the in-band copy overwrites the rest.

        The ~1us Pool gather must not overlap the DVE gate window (its
        SBUF traffic slowed concurrent DVE gate math ~3x), so `dep` pins
        it: a 1-element Pool read of h_{t-2} delays the copy until the
        previous step's h-muls are done -- the engine-idle dead zone."""
        xa = A2b[64:128, :]
        lo, hi = band(t)
        n = hi - lo + 1
        if dep is not None:
            nc.gpsimd.tensor_copy(out=SCR[:, 0:1], in_=dep[0:1, 12:13])
        if full_zero:
            nc.gpsimd.memset(xa, 0.0)
        else:
            lo6 = band(t - 6)[0]
            if lo > lo6:
                nc.gpsimd.memset(xa[:, 4 * lo6:4 * lo], 0.0)
        nc.gpsimd.tensor_copy(
            out=v(xa, 4 * lo, [[4, n], [1, 4]]),
            in_=v(IN_ap, 63 * lo + t, [[63, n], [4096, 4]]),
        )

    # ---------------- initial state ----------------
    # Four persistent A2 buffers used round-robin (not pool-rotated: the
    # same tensor identity lets xprep skip re-zeroing cols that an earlier
    # pass of the same buffer already zeroed, and avoids pool-slot FIFO
    # serialization).
    A2T = [big.tile([128, 256], BF, tag=f"A2buf{i}", name=f"A2buf{i}")
           for i in range(6)]
    A2 = A2T[0]
    nc.gpsimd.memset(A2[0:64, :], 0.0)
    xprep(A2, 0)
    # bf16 halves of c-state (matmul rhs; both re-based to partition 0 --
    # matmul rhs must share the lhsT's base partition)
    C2e = st.tile([64, 128], BF, tag="C2e", bufs=3)
    nc.gpsimd.memset(C2e[:, :], 0.0)
    C2o = st.tile([64, 128], BF, tag="C2o", bufs=3)
    nc.gpsimd.memset(C2o[:, :], 0.0)

    Uprev = None   # (U psum tile, t) pending upsample bias-add
    Hprev = None   # A2 tile holding h_{t-1} (rhs of this step)

    def emit_upsample(A2h, t):
        """Upsample matmul on the in-band part of h_t (held in A2h)."""
        U = ps.tile([128, 256], F32, tag="U")
        lo, hi = band(t)
        n = hi - lo + 1
        nc.tensor.matmul(
            U[:, 4 * lo:4 * (hi + 1)],
            LU[:, :],
            A2h[0:64, 4 * lo:4 * (hi + 1)],
            start=True, stop=True,
        )
        return U

    def emit_outadd(U, t):
        """OUT bias-add on the Scalar engine (Identity activation with a
        per-partition bias AP).  On DVE this op kept executing mid-window
        and blocked the gate math behind it in queue order; ACT has idle
        budget after the tanhs and can read PSUM directly."""
        lo, hi = band(t)
        n = hi - lo + 1
        nc.scalar.activation(
            v(OUT_ap, 63 * lo + t, [[4096, 4], [63, n]]),
            v(U[:, :], 4 * lo, [[1, 4], [4, n]]),
            AF.Identity, bias=bup[:, 0:1],
        )

    def store_block(p0, np_):
        """DMA OUT rows [p0, p0+np_) to DRAM (all images, all channels)."""
        for b in range(B):
            nc.sync.dma_start(
                out=dv(out_d, b * 128 * H * W + p0 * W, [[4096, 128], [1, np_ * W]]),
                in_=OUT[:, b * H * W + p0 * W: b * H * W + (p0 + np_) * W],
            )

    # ---------------- the recurrence ----------------
    for t in range(NW):
        # -- Pool: x for step t+1, pinned to the dead zone after the
        #    previous step's h-muls (tiny dep read of h_{t-1}); emitted
        #    FIRST so its WAR semaphore threshold is recorded against the
        #    previous iteration's PE counter, not this one's --
        A2n = A2T[(t + 1) % 6]
        if t + 1 < NW:
            xprep(A2n, t + 1, full_zero=(t + 1 <= 5), dep=A2)

        # -- PE: c2c matmuls FIRST: their rhs (the bf16 c casts) lands
        #    ~600ns before h_{t-1} completes, so they fill the PE idle
        #    window ahead of the gate matmuls and Cp is ready well before
        #    STT_e needs it --
        Cp = ps.tile([128, 128], F32, tag="Cp")
        for w in (0, 1):
            cl = slice(64 * w, 64 * w + 64)
            nc.tensor.matmul(Cp[0:64, cl], LC1[:, :], C2e[:, cl],
                             start=True, stop=False, skip_group_check=True)
            nc.tensor.matmul(Cp[64:128, cl], LC1[:, :], C2o[:, cl],
                             start=True, stop=False, skip_group_check=True)
            # u'=1 out += Wc0 @ c-even (same kap)
            nc.tensor.matmul(Cp[64:128, cl], LC0[:, :], C2e[:, cl],
                             start=False, stop=True, skip_group_check=True)
            # u'=0 out += Wc0 @ c-odd at kap-1:
            if w == 0:
                # kap = 2j (j>=1) <- kap-1 = 2(j-1)+1: w=1 half, j-1
                nc.tensor.matmul(
                    v(Cp[0:64, :], 1, [[16, 4], [1, 15]]),
                    LC0[:, :], v(C2o[:, :], 64, [[16, 4], [1, 15]]),
                    start=False, stop=True, skip_group_check=True,
                )
            else:
                # kap = 2j+1 <- kap-1 = 2j: w=0 half, same j
                nc.tensor.matmul(
                    v(Cp[0:64, :], 64, [[16, 4], [1, 16]]),
                    LC0[:, :], v(C2o[:, :], 0, [[16, 4], [1, 16]]),
                    start=False, stop=True, skip_group_check=True,
                )

        # -- PE: gate matmuls (critical path) --
        P01 = ps.tile([128, 256], F32, tag="P01")
        P23 = ps.tile([128, 256], F32, tag="P23")
        for P, LA, LB in ((P01, LA01, LB01), (P23, LA23, LB23)):
            nc.tensor.matmul(P[:, :], LA[:, :], A2[:, :], start=True, stop=False)
            # Ws0 row-shift tap: out (b, p>=1) += Ws0 @ h[(b, p-1)]
            # (b-minor packing makes the row shift a flat column shift)
            nc.tensor.matmul(
                P[:, 4:256],
                LB[:, :],
                A2[0:64, 0:252],
                start=False, stop=True,
            )

        # -- PE: upsample of the previous step (off critical path) --
        if Hprev is not None:
            Uprev = (emit_upsample(Hprev, t - 1), t - 1)
            Hprev = None

        # -- ACT: the two sigmoid scatters P -> G (contiguous G halves) --
        G = tmp.tile([128, 512], BF, tag="G")
        Gap = G[:, :]
        for w, (P, bsg) in ((0, (P01, bsg01)), (1, (P23, bsg23))):
            nc.scalar.activation(
                v(Gap, 256 * w, [[64, 4], [16, 4], [1, 16]]),
                v(P[:, :], 0, [[64, 4], [1, 4], [4, 16]]),
                AF.Sigmoid, bias=bsg[:, 0:1],
            )


        # -- DVE gate math + ACT tanh, even stream first (runs in the
        #    shadow of sigmoid 2); all operands are contiguous slices --
        T1 = tmp.tile([128, 128], F32, tag="T1")
        T2 = tmp.tile([128, 128], F32, tag="T2")
        C2n = tmp.tile([128, 128], F32, tag="C2n")
        TH = tmp.tile([128, 128], BF, tag="TH")
        for w in (0, 1):
            g0 = 256 * w
            cl = slice(64 * w, 64 * w + 64)
            nc.vector.tensor_mul(T1[:, cl], G[:, g0:g0 + 64], G[:, g0 + 64:g0 + 128])
            nc.vector.scalar_tensor_tensor(
                out=T2[:, cl], in0=Cp[:, cl], scalar=bc2c2[:, 0:1],
                in1=G[:, g0 + 128:g0 + 192], op0=ALU.add, op1=ALU.mult,
            )
            nc.vector.tensor_add(C2n[:, cl], T1[:, cl], T2[:, cl])
            nc.scalar.activation(TH[:, cl], C2n[:, cl], AF.Tanh)
        # -- DVE: bf16 casts of c for the next c2c, emitted BEFORE the
        #    h-muls: they fill the DVE gap while tanh_o runs on ACT --
        C2en = st.tile([64, 128], BF, tag="C2e", bufs=3)
        nc.vector.tensor_copy(C2en[:, :], C2n[0:64, :])
        C2on = st.tile([64, 128], BF, tag="C2o", bufs=3)
        nc.vector.tensor_copy(C2on[:, :], C2n[64:128, :])
        # h_t = og * tanh(c_t) -> A2n[0:64], col 4*p2 + b, p2 = 4j + 2w + u
        for w in (0, 1):
            for u in (0, 1):
                nc.vector.tensor_mul(
                    v(A2n[0:64, :], 8 * w + 4 * u, [[16, 16], [1, 4]]),
                    v(G[64 * u:64 * u + 64, :], 256 * w + 192, [[1, 16], [16, 4]]),
                    v(TH[64 * u:64 * u + 64, :], 64 * w, [[1, 16], [16, 4]]),
                )

        if Uprev is not None:
            emit_outadd(*Uprev)
            Uprev = None

        if STAGGER_OUT and t >= 80 and (t - 80) % 16 == 0 and (t - 80) // 16 < 3:
            store_block(16 * ((t - 80) // 16), 16)

        A2 = A2n
        Hprev = A2n
        C2e = C2en
        C2o = C2on

    # ---------------- epilogue: last upsample + store ----------------
    U = emit_upsample(A2, NW - 1)
    emit_outadd(U, NW - 1)
    if STAGGER_OUT:
        store_block(48, 16)
    else:
        store_block(0, 64)


def build_nc():
    nc = bacc.Bacc("TRN2", target_bir_lowering=False, debug=False)
    ins = {
        "inputs": nc.dram_tensor("inputs", [B, C, H, W], BF, kind="ExternalInput").ap(),
        "w_i2s": nc.dram_tensor("w_i2s", [4 * HID, C], F32, kind="ExternalInput").ap(),
        "b_i2s": nc.dram_tensor("b_i2s", [4 * HID], F32, kind="ExternalInput").ap(),
        "w_s2s": nc.dram_tensor("w_s2s", [4 * HID, HID, 2], F32, kind="ExternalInput").ap(),
        "b_s2s": nc.dram_tensor("b_s2s", [4 * HID], F32, kind="ExternalInput").ap(),
        "w_c2c": nc.dram_tensor("w_c2c", [HID, HID, 2], F32, kind="ExternalInput").ap(),
        "b_c2c": nc.dram_tensor("b_c2c", [HID], F32, kind="ExternalInput").ap(),
        "w_up": nc.dram_tensor("w_up", [2 * HID, HID], F32, kind="ExternalInput").ap(),
        "b_up": nc.dram_tensor("b_up", [2 * HID], F32, kind="ExternalInput").ap(),
    }
    outs = {"out": nc.dram_tensor("out", [B, 2 * HID, H, W], F32, kind="ExternalOutput").ap()}
    with tile.TileContext(nc) as tc:
        with ExitStack() as ctx:
            build_kernel(ctx, tc, outs, ins)
    nc.compile()
    return nc


# ---------------------------------------------------------------------------
# Harness entry point: full inputs -> shard over 8 cores -> full output.
# ---------------------------------------------------------------------------
import ml_dtypes
from concourse.bass_utils import run_bass_kernel_spmd

N_CORES = 8
TRACE = False
LAST_EXEC_NS = None
LAST_RESULT = None
_NC = None


def _get_nc():
    global _NC
    if _NC is None:
        _NC = build_nc()
    return _NC


def kernel(**inputs):
    global LAST_EXEC_NS, LAST_RESULT
    nc = _get_nc()
    full = {k: np.ascontiguousarray(np.asarray(val, np.float32))
            for k, val in inputs.items()}
    xs = full["inputs"].astype(ml_dtypes.bfloat16)
    in_maps = []
    for i in range(N_CORES):
        m = dict(full)
        m["inputs"] = np.ascontiguousarray(xs[B * i:B * (i + 1)])
        in_maps.append(m)
    res = run_bass_kernel_spmd(nc, in_maps, list(range(N_CORES)), trace=TRACE)
    LAST_EXEC_NS = res.exec_time_ns
    LAST_RESULT = res
    return np.concatenate([res.results[i]["out"] for i in range(N_CORES)], axis=0)
